# revision 51
# baseline (speedup 1.0000x reference)
"""TRN2 Bass/Tile kernel for nn_DHSMBlock (SSM + self-attn + hierarchical memory + FFN).

Sharding: data-parallel over batch. B=8 rows -> 8 NeuronCores, one row per core,
no collectives. Each core gets the full weight set (host pre-transposed, bf16).

On-device layout is feature-major: every activation lives as X^T [feature, token]
so all matmuls contract over the partition dim. All matmul operands are bf16
(fp32 PSUM accumulation); fp32 is kept only for the SSM scan and LN statistics
rows. LayerNorm stats are ones-vector matmuls on the PE; rstd comes from an
ACT-engine Rsqrt; [1,T] -> [P,T] broadcasts go through a DRAM round-trip.
Self-attention score matmuls are row-packed two-heads-at-a-time (K=64 each) so
they run concurrently in disjoint PE row groups. The hierarchical-memory
compressors only depend on mem inputs, so they are emitted interleaved with the
SSM stage to cover its serial sections. Weight DMAs are spread over the SP/ACT
HW-DGE queues and the Pool SW-DGE queue, ordered by first use.
"""

import os
from contextlib import ExitStack

import numpy as np

os.environ.setdefault("MYCRO_LOCAL_CACHE", "1")

import concourse.bass as bass
import concourse.mybir as mybir
import concourse.tile as tile
from concourse import bass_utils
from concourse.masks import make_identity

F32 = mybir.dt.float32
BF16 = mybir.dt.bfloat16
AF = mybir.ActivationFunctionType
OP = mybir.AluOpType

B, T, H, S = 8, 1024, 1024, 128
NH, DH = 16, 64          # self-attention heads
RH, RDH = 4, 256         # retriever heads
COMP = [1024, 512, 256]  # compressor widths
P = 128
HT = H // P              # 8 feature tiles
NCH = 512                # matmul moving-dim chunk (one fp32 PSUM bank)
EPS = 1e-5

# packed per-partition vectors: (name, rows); layout (k p) -> [p, k]
VDEFS = [
    ("A", S), ("sg_b", S), ("Dp1", H), ("outp_b", H), ("a_bq", H), ("a_const", H),
    ("r_bq", H), ("r_const", H), ("mg_b", H), ("f_b1", 4 * H), ("f_b2", H),
    ("sln_g", H), ("sln_b", H), ("n1_g", H), ("n1_b", H),
    ("n2_g", H), ("n2_b", H), ("n3_g", H), ("n3_b", H),
    ("c0_b1", 1024), ("c1_b1", 512), ("c2_b1", 256),
    ("c0_b2", H), ("c1_b2", H), ("c2_b2", H),
]
VOFF = {}
_o = 0
for _n, _r in VDEFS:
    VOFF[_n] = _o
    _o += _r // P
VCOLS = _o


def build_nc(fl):
    nc = bass.Bass("TRN2", target_bir_lowering=False, debug=False, num_devices=8)
    D = {}

    def din(name, shape, dt=BF16):
        D[name] = nc.dram_tensor(name, list(shape), dt, kind="ExternalInput").ap()

    din("xT", (H, T))
    for i in range(3):
        din(f"m{i}T", (H, 256))
    din("wsgT", (H, S)); din("wBT", (H, S)); din("wCT", (S, H))
    din("vpack", (P, VCOLS), F32)
    din("outp_wT", (H, H))
    din("a_wqT", (H, H)); din("a_wkT", (H, H)); din("a_wvT", (H, H))
    din("a_woT", (H, H))
    for i, c in enumerate(COMP):
        din(f"c{i}_w1T", (H, c)); din(f"c{i}_w2T", (c, H))
    din("r_wqT", (H, H)); din("r_wkT", (H, H)); din("r_wvT", (H, H))
    din("r_woT", (H, H))
    din("mg_wT", (2 * H, H))
    din("f_w1T", (H, 4 * H)); din("f_w2T", (4 * H, H))
    out_d = nc.dram_tensor("out", [T, H], F32, kind="ExternalOutput").ap()

    with tile.TileContext(nc, pool_alloc_mode="queue") as tc:
        _body(nc, tc, D, out_d, fl)
    _split_matmul_waits(nc)
    return nc


_WAIT_EXEMPT = {
    "InstEventSemaphore", "InstAllEngineBarrier",
    "InstUnconditionalBranch", "InstCompareAndBranch", "InstIndirectBranch",
    "InstHalt", "InstBranchHint",
}


def _split_matmul_waits(nc):
    """TPB engine instruction encodings carry at most one sync wait; move
    surplus waits onto a preceding same-engine no-op (sequencer WAITs)."""
    import bass_rust
    cnt = 0
    for f in nc.m.functions:
        for blk in f.blocks:
            insts = blk.instructions
            out = []
            changed = False
            for inst in insts:
                if (type(inst).__name__ not in _WAIT_EXEMPT
                        and not isinstance(inst, bass_rust.InstISA)):
                    si = inst.sync_info
                    if si is not None and len(si.on_wait) > 1:
                        surplus = list(si.on_wait[:-1])
                        for j in range(0, len(surplus), 2):
                            ev = bass_rust.InstEventSemaphore(name=f"I-wsplit-{cnt}")
                            cnt += 1
                            ev.engine = inst.engine
                            ev.bass_nofuse = True
                            ev.sync_info = bass_rust.SyncInfo(
                                on_wait=surplus[j:j + 2], on_update=[])
                            out.append(ev)
                        inst.sync_info = bass_rust.SyncInfo(
                            on_wait=[si.on_wait[-1]], on_update=list(si.on_update))
                        changed = True
                out.append(inst)
            if changed:
                blk.instructions = out
    return nc


def _body(nc, tc, D, out_d, fl):
    import itertools
    _bc_ctr = itertools.count()
    ctx = ExitStack()

    # ---------- ambient pools ----------
    pv = ctx.enter_context(tc.tile_pool(name="pv", bufs=1))
    resid = ctx.enter_context(tc.tile_pool(name="resid", bufs=2))
    dscr = ctx.enter_context(tc.tile_pool(name="dscr", bufs=4, space="DRAM"))
    # long-lived mid-stage pools, entered in reverse close order:
    # pKr/pVr/pQr close at the end of the retriever attention; pc (chat)
    # closes after the Kr/Vr projections.
    cstk2 = ExitStack()
    pKr = cstk2.enter_context(tc.tile_pool(name="pKr", bufs=1))
    pVr = cstk2.enter_context(tc.tile_pool(name="pVr", bufs=1))
    pQr = cstk2.enter_context(tc.tile_pool(name="pQr", bufs=1))
    pc_stk = ExitStack()
    pc = pc_stk.enter_context(tc.tile_pool(name="pc", bufs=1))

    def bcast(dst_ap, src_ap, parts, tn, tag, dt=BF16):
        """Broadcast a [1,tn] SBUF row to [parts,tn] via a DRAM round-trip."""
        scr = dscr.tile([1, tn], dt, tag=tag, name=f"scr_{tag}_{next(_bc_ctr)}")
        nc.sync.dma_start(out=scr[:], in_=src_ap)
        nc.sync.dma_start(out=dst_ap, in_=scr[0:1, :].broadcast_to((parts, tn)))

    def rtile(k, name):
        return resid.tile([P, T], BF16, tag=f"r{k}", name=name)

    # packed vectors (one DMA)
    vpk = pv.tile([P, VCOLS], F32, tag="vpk")
    nc.scalar.dma_start(out=vpk[:], in_=D["vpack"][:, :])

    def vcol(nm, k=0):
        o = VOFF[nm] + k
        return vpk[:, o:o + 1]

    # all-ones [P, P] stationary: partition-sum matmuls whose outputs land
    # broadcast on all 128 partitions AND that count as full-array activity
    # for the PE HAM clock gate (M=1 ones-column matmuls read as idle and
    # keep the PE throttled at 1.2 GHz).
    allones = pv.tile([P, P], BF16, tag="allones")
    nc.vector.memset(allones[:], 1.0)
    eps_t = pv.tile([P, 1], F32, tag="eps")
    nc.vector.memset(eps_t[:], EPS)
    identb = pv.tile([P, P], BF16, tag="identb")
    make_identity(nc, identb[:])

    xs = []
    for k in range(HT):
        t = rtile(k, f"x_{k}")
        nc.sync.dma_start(out=t[:], in_=D["xT"][k * P:(k + 1) * P, :])
        xs.append(t)

    # ---------- helpers ----------
    def mm(ps, steps, nch=NCH):
        """ps[M,N] = sum_k steps[k].lhsT.T @ steps[k].rhs ; chunks the moving dim."""
        n = ps.shape[-1]
        K = len(steps)
        for c0 in range(0, n, nch):
            ce = min(c0 + nch, n)
            for k, (lt, rt) in enumerate(steps):
                nc.tensor.matmul(ps[:, c0:ce], lt, rt[:, c0:ce],
                                 start=(k == 0), stop=(k == K - 1))

    def load_wblocks(pool, dram_ap, nk, cols, tag, c0=0, bufs=1, eng=None,
                     tile_cols=None):
        eng = eng or nc.scalar
        tiles = []
        for k in range(nk):
            t = pool.tile([P, tile_cols or cols], BF16, tag=f"{tag}{k}", bufs=bufs,
                          name=f"{tag}{k}_{c0}")
            eng.dma_start(out=t[:, 0:cols],
                          in_=dram_ap[k * P:(k + 1) * P, c0:c0 + cols])
            tiles.append(t)
        return tiles

    def preload_w(pool, wname, tag, nk=HT, mh=4, eng=None, halves=(0, 1)):
        """Early weight loads for a later proj(): emitting the DMAs from a
        pool created before the current stage's pools keeps them out of the
        SBUF ring's wait-for-free chain at the stage boundary."""
        return {half: load_wblocks(pool, D[wname], nk, mh * P, tag,
                                   c0=half * mh * P, bufs=1, eng=eng)
                for half in halves}

    def proj(wname, rhs_tiles, epilogue, pool, ppool, tag, nk=HT, mh=4, wbufs=2,
             eng=None, pre=None, ksplit=None):
        """out[m] = epilogue(m, psum(W^T[:,m] @ rhs)), streaming W in col-halves.

        ksplit=j: accumulate k<j for every m of the group first, then finish
        k>=j -- covers late-arriving rhs tiles (attention epilogue tails)
        with ~mh*j*2 matmuls of PE work instead of stalling."""
        for half in range(HT // mh):
            if pre is not None and half in pre:
                wb = pre[half]
            else:
                wb = load_wblocks(pool, D[wname], nk, mh * P, tag,
                                  c0=half * mh * P, bufs=wbufs, eng=eng)
            if ksplit is None:
                for ml in range(mh):
                    m = half * mh + ml
                    ps = ppool.tile([P, T], F32, tag="pbig", name=f"{tag}ps{m}")
                    mm(ps, [(wb[k][:, ml * P:(ml + 1) * P], rhs_tiles[k][:])
                            for k in range(nk)])
                    epilogue(m, ps)
            else:
                pss = []
                for ml in range(mh):
                    m = half * mh + ml
                    ps = ppool.tile([P, T], F32, tag=f"pb{ml}",
                                    name=f"{tag}ps{m}")
                    for c0 in range(0, T, NCH):
                        for k in range(ksplit):
                            nc.tensor.matmul(ps[:, c0:c0 + NCH],
                                             wb[k][:, ml * P:(ml + 1) * P],
                                             rhs_tiles[k][:, c0:c0 + NCH],
                                             start=(k == 0), stop=False)
                    pss.append(ps)
                for ml in range(mh):
                    m = half * mh + ml
                    ps = pss[ml]
                    for c0 in range(0, T, NCH):
                        for k in range(ksplit, nk):
                            nc.tensor.matmul(ps[:, c0:c0 + NCH],
                                             wb[k][:, ml * P:(ml + 1) * P],
                                             rhs_tiles[k][:, c0:c0 + NCH],
                                             start=False, stop=(k == nk - 1))
                    epilogue(m, ps)

    def layer_norm(z, gname, pools, mk_out, Tn=T, nch=NCH):
        """Feature-dim (partition) LN over bf16 z tiles. Stats come from
        all-ones [P,P] matmuls, so mean/var land already broadcast on all
        partitions -- no [1,T] row ops and no DRAM broadcast round-trip.
        mk_out(k) -> bf16 tile."""
        pp_stat, lnaux, _ = pools
        nchunk = max(1, Tn // nch)
        cw = min(Tn, nch)
        u = next(_bc_ctr)
        outs = [mk_out(k) for k in range(HT)]
        for c in range(nchunk):
            cs = slice(c * cw, (c + 1) * cw)
            ps_s = pp_stat.tile([P, cw], F32, tag="st",
                                name=f"lnps_s{c}_{u}")
            ps_q = pp_stat.tile([P, cw], F32, tag="st",
                                name=f"lnps_q{c}_{u}")
            for k in range(HT):
                nc.tensor.matmul(ps_s[:, :], allones[:, :], z[k][:, cs],
                                 start=(k == 0), stop=(k == HT - 1))
            for k in range(HT):
                sq = lnaux.tile([P, cw], BF16, tag="lnsq", bufs=3)
                # z^2 on ACT (Square is in every table; ACT is the least
                # loaded engine at every LN site)
                if k % 4 == 3:
                    nc.gpsimd.tensor_mul(sq[:], z[k][:, cs], z[k][:, cs])
                else:
                    nc.scalar.activation(sq[:], z[k][:, cs], AF.Square)
                nc.tensor.matmul(ps_q[:, :], allones[:, :], sq[:, :],
                                 start=(k == 0), stop=(k == HT - 1))
            mean_c = lnaux.tile([P, cw], F32, tag="mean", bufs=1,
                                name=f"ln_mean{c}_{u}")
            nc.scalar.activation(mean_c[:], ps_s[:], AF.Copy, bias=0.0,
                                 scale=1.0 / H)
            m2 = lnaux.tile([P, cw], F32, tag="m2", bufs=1, name=f"ln_m2{c}_{u}")
            nc.vector.tensor_mul(m2[:], mean_c[:], mean_c[:])
            var_c = lnaux.tile([P, cw], F32, tag="var", bufs=1,
                               name=f"ln_var{c}_{u}")
            nc.vector.scalar_tensor_tensor(out=var_c[:], in0=ps_q[:],
                                           scalar=1.0 / H, in1=m2[:],
                                           op0=OP.mult, op1=OP.subtract)
            # rstd = exp(-0.5*ln(var+eps)): stays in the Ln/Exp act table
            # used by every softmax in the kernel -- no act-table switches.
            lnv = lnaux.tile([P, cw], F32, tag="lnv", bufs=1,
                             name=f"ln_lnv{c}_{u}")
            nc.scalar.activation(lnv[:], var_c[:], AF.Ln, bias=eps_t[:, 0:1])
            rstd_c = lnaux.tile([P, cw], BF16, tag="rstd", bufs=2,
                                name=f"ln_rstd{c}_{u}")
            nc.scalar.activation(rstd_c[:], lnv[:], AF.Exp, scale=-0.5)
            mr_c = lnaux.tile([P, cw], BF16, tag="mr", bufs=2,
                              name=f"ln_mr{c}_{u}")
            nc.vector.tensor_mul(mr_c[:], mean_c[:], rstd_c[:])
            for k in range(HT):
                o = outs[k]
                # weighted split: DVE ~1.6x faster than Pool per op
                eng = nc.vector if k < 6 else nc.gpsimd
                eng.tensor_mul(o[:, cs], z[k][:, cs], rstd_c[:])
                eng.tensor_sub(o[:, cs], o[:, cs], mr_c[:])
                if not fl[f"{gname}_trivial"]:
                    nc.vector.tensor_scalar(out=o[:, cs], in0=o[:, cs],
                                            scalar1=vcol(f"{gname}_g", k),
                                            scalar2=vcol(f"{gname}_b", k),
                                            op0=OP.mult, op1=OP.add)
        return outs

    # =========================================================================
    # Stage A: SSM layer, emission interleaved with the hierarchical-memory
    # compressors (which only depend on mem inputs) to cover serial sections.
    # =========================================================================
    ssm = ExitStack()
    ssm1 = ssm.enter_context(tc.tile_pool(name="ssm1", bufs=1))
    comp = ssm.enter_context(tc.tile_pool(name="comp", bufs=1))
    ppc = ssm.enter_context(tc.tile_pool(name="ppc", bufs=2, space="PSUM"))
    lnz = ssm.enter_context(tc.tile_pool(name="lnzA", bufs=8))
    lnaux = ssm.enter_context(tc.tile_pool(name="lnauxA", bufs=1))
    pbc = ssm.enter_context(tc.tile_pool(name="bcA", bufs=1))
    lnout = ssm.enter_context(tc.tile_pool(name="lnoA", bufs=8))
    ppst = ssm.enter_context(tc.tile_pool(name="ppstA", bufs=2, space="PSUM"))

    wsg = load_wblocks(ssm1, D["wsgT"], HT, S, "wsg", eng=nc.scalar)
    wB = load_wblocks(ssm1, D["wBT"], HT, S, "wB", eng=nc.scalar)
    wC = ssm1.tile([S, H], BF16, tag="wC")
    nc.scalar.dma_start(out=wC[:], in_=D["wCT"][:, :])

    chat = {}
    _comp_mid = {}

    def comp_mid(i):
        cwid = COMP[i]
        ct = cwid // P
        mT = load_wblocks(comp, D[f"m{i}T"], HT, 256, "mT", bufs=1, eng=nc.sync)
        mid = []
        for h0 in range(0, ct, 4):
            hc = min(4, ct - h0)
            w1 = load_wblocks(comp, D[f"c{i}_w1T"], HT, hc * P, "cw1_", bufs=2,
                              c0=h0 * P, tile_cols=512, eng=nc.scalar)
            for cm in range(hc):
                ps = ppc.tile([P, 256], F32, tag="pc1", name=f"pm{i}_{h0 + cm}")
                mm(ps, [(w1[k][:, cm * P:(cm + 1) * P], mT[k][:])
                        for k in range(HT)])
                md = comp.tile([P, 256], BF16, tag=f"mid{h0 + cm}",
                               name=f"mid{i}_{h0 + cm}")
                nc.scalar.activation(md[:], ps[:], AF.Relu,
                                     bias=vcol(f"c{i}_b1", h0 + cm))
                mid.append(md)
        _comp_mid[i] = mid

    def comp_chat(i):
        mid = _comp_mid.pop(i)
        ct = COMP[i] // P
        chat[i] = []
        for h0 in range(2):
            w2 = load_wblocks(comp, D[f"c{i}_w2T"], ct, 512, "cw2_", bufs=2,
                              c0=h0 * 512, tile_cols=512, eng=nc.scalar)
            for ml in range(4):
                m = h0 * 4 + ml
                ps = ppc.tile([P, 256], F32, tag="pc1", name=f"pc{i}_{m}")
                mm(ps, [(w2[k][:, ml * P:(ml + 1) * P], mid[k][:])
                        for k in range(ct)])
                cm_t = pc.tile([P, 256], BF16, tag=f"c{i}_{m}", name=f"c{i}_{m}")
                if fl[f"c{i}_b2_zero"]:
                    nc.scalar.activation(cm_t[:], ps[:], AF.Copy)
                else:
                    nc.vector.tensor_scalar_add(cm_t[:], ps[:],
                                                vcol(f"c{i}_b2", m))
                chat[i].append(cm_t)

    def kr_proj(i):
        """Retriever K projection of compressed memory i (fills LN gaps)."""
        Kr[i] = []
        with tc.tile_pool(name=f"rkw{i}", bufs=1) as rkw:
            for hf in range(2):
                wk = load_wblocks(rkw, D["r_wkT"], HT, 512, "wkr",
                                  c0=hf * 512, bufs=1, eng=nc.sync)
                for ml in range(4):
                    m = hf * 4 + ml
                    ps = ppc.tile([P, 256], F32, tag="pc1", name=f"pk{i}_{m}")
                    mm(ps, [(wk[k][:, ml * P:(ml + 1) * P], chat[i][k][:])
                            for k in range(HT)])
                    kt_ = pKr.tile([P, 256], BF16, tag=f"kr{i}_{m}",
                                   name=f"kr{i}_{m}")
                    nc.scalar.activation(kt_[:], ps[:], AF.Copy)
                    Kr[i].append(kt_)

    def vr_proj(idxs, tag):
        """Retriever V projection of compressed memories (fills LN gaps)."""
        with tc.tile_pool(name=f"rvw{tag}", bufs=1) as rvw, \
             tc.tile_pool(name=f"ppvr{tag}", bufs=2, space="PSUM") as ppvr:
            wvr = load_wblocks(rvw, D["r_wvT"], HT, H, "wvr", eng=nc.scalar)
            for i in idxs:
                Vr[i] = []
                for kvt in range(2):
                    ps = ppvr.tile([P, T], F32, tag="pkv2",
                                   name=f"pv{i}_{kvt}")
                    mm(ps, [(chat[i][k][:, kvt * P:(kvt + 1) * P], wvr[k][:])
                            for k in range(HT)])
                    vt = pVr.tile([P, T], BF16, tag=f"vr{i}_{kvt}",
                                  name=f"vr{i}_{kvt}")
                    nc.scalar.activation(vt[:], ps[:], AF.Copy)
                    Vr[i].append(vt)

    Kr, Vr = {}, {}
    z1 = []
    with tc.tile_pool(name="ppA", bufs=2, space="PSUM") as ppA:
        psG = ppA.tile([P, T], F32, tag="pbig")
        mm(psG, [(wsg[k][:], xs[k][:]) for k in range(HT)])
        gate = ssm1.tile([P, T], BF16, tag="gate")
        nc.scalar.activation(gate[:], psG[:], AF.Sigmoid, bias=vcol("sg_b"))
        psB = ppA.tile([P, T], F32, tag="pbig")
        mm(psB, [(wB[k][:], xs[k][:]) for k in range(HT)])
        u = ssm1.tile([P, T], F32, tag="u")
        nc.vector.tensor_mul(u[:], psB[:], gate[:])
        states = ssm1.tile([P, T], F32, tag="states")
        nc.vector.tensor_tensor_scan(states[:], vcol("A").to_broadcast((P, T)),
                                     u[:], 0.0, op0=OP.mult, op1=OP.add)
        states_bf = ssm1.tile([P, T], BF16, tag="states_bf")
        nc.vector.tensor_copy(out=states_bf[:], in_=states[:])

        comp_mid(0)  # covers the scan + z1 serial section

        for m in range(HT):
            psY = ppA.tile([P, T], F32, tag="pbig", name=f"psY{m}")
            mm(psY, [(wC[:, m * P:(m + 1) * P], states_bf[:])])
            zm = lnz.tile([P, T], BF16, tag="z", name=f"z1_{m}")
            nc.vector.scalar_tensor_tensor(out=zm[:], in0=xs[m][:],
                                           scalar=vcol("Dp1", m), in1=psY[:],
                                           op0=OP.mult, op1=OP.add)
            z1.append(zm)

    comp_chat(0)

    # prefetch outp weights so they are resident when the sln LN completes
    woustk = ExitStack()
    wou = woustk.enter_context(tc.tile_pool(name="wouA", bufs=1))
    wou_pre = preload_w(wou, "outp_wT", "wou", halves=(0,), eng=nc.sync)

    ln1 = layer_norm(z1, "sln", (ppst, lnaux, pbc),
                     lambda k: lnout.tile([P, T], BF16, tag="ln1", name=f"ln1_{k}"))

    kr_proj(0)  # covers the sln LN serial chain
    comp_mid(1)

    z2 = []

    def ep_outp(m, ps):
        zm = lnz.tile([P, T], BF16, tag="z", name=f"z2_{m}")
        if fl["outp_b_zero"]:
            nc.vector.tensor_add(zm[:], ps[:], xs[m][:])
        else:
            nc.vector.scalar_tensor_tensor(out=zm[:], in0=ps[:],
                                           scalar=vcol("outp_b", m),
                                           in1=xs[m][:], op0=OP.add, op1=OP.add)
        z2.append(zm)

    with tc.tile_pool(name="ppA2", bufs=2, space="PSUM") as ppA2:
        proj("outp_wT", ln1, ep_outp, wou, ppA2, "wou", wbufs=1,
             eng=nc.gpsimd, pre=wou_pre)
    woustk.close()

    comp_chat(1)

    x1 = layer_norm(z2, "n1", (ppst, lnaux, pbc), lambda k: rtile(k, f"x1_{k}"))

    kr_proj(1)  # covers the n1 LN serial chain
    comp_mid(2)
    comp_chat(2)
    kr_proj(2)
    ssm.close()
    # prefetch pools, opened here so their ring slots precede the attention
    # pools (LIFO close order: wq0 closes after bstk, wpo after rwq).
    wostk = ExitStack()
    wpo = wostk.enter_context(tc.tile_pool(name="woB", bufs=1))
    wo_pre = preload_w(wpo, "a_woT", "wo", mh=2, halves=(0, 1, 2, 3),
                       eng=nc.scalar)
    wqstk = ExitStack()
    wq0 = wqstk.enter_context(tc.tile_pool(name="wq0", bufs=1))
    wq_pre = preload_w(wq0, "a_wqT", "wq0", halves=(0,), eng=nc.sync)
    vr_proj([0, 1], "a")  # covers the n1->QKV transition

    # =========================================================================
    # Stage B: self-attention. One head at a time; every matmul uses the full
    # 128x128 PE array (zero-padded K rows / V columns) so the HAM clock gate
    # sees the PE as busy and keeps it at 2.4 GHz -- partial-array matmuls
    # (K=64 scores, M=65 AV) left the whole attention region throttled at
    # 1.2 GHz.
    # =========================================================================
    Oh = [rtile(g, f"oh{g}") for g in range(HT)]
    bstk = ExitStack()
    pQ = bstk.enter_context(tc.tile_pool(name="pQ", bufs=1))
    pK = bstk.enter_context(tc.tile_pool(name="pK", bufs=1))
    pV = bstk.enter_context(tc.tile_pool(name="pV", bufs=1))
    # Kz[h]: [P, T] with the other head's 64 feature rows zeroed -> K=128
    # score matmuls with a full Q rhs.
    Kz = [pK.tile([P, T], BF16, tag=f"kz{h}", name=f"kz{h}")
          for h in range(NH)]
    for h in range(NH):
        zs = slice(DH, P) if (h % 2 == 0) else slice(0, DH)
        (nc.vector if h % 2 == 0 else nc.gpsimd).memset(Kz[h][zs, :], 0.0)
    with tc.tile_pool(name="wqkv", bufs=1) as wqkv, \
         tc.tile_pool(name="ppB1", bufs=2, space="PSUM") as ppB1:
        Qh, Vp = [], []

        def ep_q(m, ps):
            qm = pQ.tile([P, T], BF16, tag=f"q{m}", name=f"q{m}")
            if fl["a_bq_zero"]:
                nc.scalar.activation(qm[:], ps[:], AF.Copy)
            else:
                nc.vector.tensor_scalar_add(qm[:], ps[:], vcol("a_bq", m))
            Qh.append(qm)

        def ep_k(m, ps):
            nc.vector.tensor_copy(out=Kz[2 * m][0:DH, :], in_=ps[0:DH, :])
            nc.scalar.activation(Kz[2 * m + 1][DH:P, :], ps[DH:P, :], AF.Copy)

        proj("a_wqT", x1, ep_q, wqkv, ppB1, "wq", wbufs=1, eng=nc.sync,
             pre=wq_pre)
        proj("a_wkT", x1, ep_k, wqkv, ppB1, "wq", wbufs=1, eng=nc.sync)
        # V token-major with a ones column appended per head (Z row)
        VW = DH + 1
        for kt in range(HT):
            vt = pV.tile([P, NH * VW], BF16, tag=f"v{kt}", name=f"v{kt}")
            vv = vt[:].rearrange("p (h c) -> p h c", c=VW)
            nc.gpsimd.memset(vv[:, :, DH:DH + 1], 1.0)
            Vp.append(vt)
        for vh in range(2):
            wvb = load_wblocks(wqkv, D["a_wvT"], HT, 512, "wq", c0=vh * 512,
                               eng=nc.scalar)
            for kt in range(HT):
                ps = ppB1.tile([P, 512], F32, tag="pvh", name=f"psV{vh}_{kt}")
                mm(ps, [(x1[k][:, kt * P:(kt + 1) * P], wvb[k][:])
                        for k in range(HT)])
                nc.vector.tensor_copy(
                    out=Vp[kt][:].rearrange("p (h c) -> p h c", c=VW)[
                        :, 8 * vh:8 * (vh + 1), 0:DH],
                    in_=ps[:].rearrange("p (h c) -> p h c", c=DH)[:, :, :])

    with tc.tile_pool(name="pexp", bufs=3) as pexp, \
         tc.tile_pool(name="poraw", bufs=5) as poraw, \
         tc.tile_pool(name="przq", bufs=1) as przq, \
         tc.tile_pool(name="prb", bufs=3) as prb, \
         tc.tile_pool(name="ppS", bufs=2, space="PSUM") as ppS, \
         tc.tile_pool(name="ppA", bufs=2, space="PSUM") as ppA:
        # Z-row groups: quads early, pairs at the end so the final 1/Z chain
        # (the wo-proj's gating input) is short; head 15's PSUM drains go to
        # ACT, which is idle after its last exp while DVE still has backlog.
        GRP = {}
        for h0, gn in ((0, 4), (4, 4), (8, 4), (12, 2), (14, 2)):
            for hh in range(h0, h0 + gn):
                GRP[hh] = (h0, gn)
        zq = None
        oraws = {}
        for h in range(NH):
            g, par = divmod(h, 2)
            psA = ppA.tile([DH + 1, T], F32, tag="pa", name=f"psA{h}")
            ets = {}
            # software pipeline: emit scores(kt), exp(kt), then AV(kt-1)
            for kt in range(HT + 1):
                if kt < HT:
                    psS = ppS.tile([P, T], F32, tag="ps", name=f"psS{h}_{kt}")
                    for c0 in range(0, T, NCH):
                        nc.tensor.matmul(psS[:, c0:c0 + NCH],
                                         Kz[h][:, kt * P:(kt + 1) * P],
                                         Qh[g][:, c0:c0 + NCH],
                                         start=True, stop=True)
                    et = pexp.tile([P, T], BF16, tag="et", name=f"et{h}_{kt}")
                    nc.scalar.activation(et[:], psS[:], AF.Exp, scale=1.0 / 8.0)
                    ets[kt] = et
                if kt > 0:
                    pkt = kt - 1
                    vs = Vp[pkt][:, h * VW:(h + 1) * VW]
                    for c0 in range(0, T, NCH):
                        nc.tensor.matmul(psA[:, c0:c0 + NCH], vs,
                                         ets[pkt][:, c0:c0 + NCH],
                                         start=(pkt == 0), stop=(pkt == HT - 1))
            # drain PSUM promptly: bf16 copy of AV rows + f32 copy of the Z
            # row into a quad-packed tile; 1/Z (Ln->Exp, same act table as
            # the softmax Exp) + DRAM-broadcast + apply happen from SBUF,
            # off the Tensor/PSUM critical path.
            # Z rows pack at 32-aligned partitions (engine outputs must start
            # at partition 0/32/64/96); the in-between rows are never read.
            g0, gn = GRP[h]
            if h == g0:
                zq = przq.tile([3 * 32 + 1, T], F32, tag="zq", bufs=1,
                               name=f"zq{g0}")
            zrow = 32 * (h - g0)
            nc.vector.tensor_copy(out=zq[zrow:zrow + 1, :],
                                  in_=psA[DH:DH + 1, :])
            if h >= NH - 2:
                # last pair: no successor reuses these PSUM banks, so apply
                # straight from PSUM and skip the bf16 drain copy
                oraws[h] = psA
            else:
                oraw = poraw.tile([DH, T], BF16, tag="or", name=f"oraw{h}")
                nc.vector.tensor_copy(out=oraw[:], in_=psA[0:DH, :])
                oraws[h] = oraw
            if h == g0 + gn - 1:
                lnq = przq.tile([3 * 32 + 1, T], F32, tag="lnq", bufs=1,
                                name=f"lnq{g0}")
                nc.scalar.activation(lnq[0:32 * (gn - 1) + 1, :],
                                     zq[0:32 * (gn - 1) + 1, :], AF.Ln)
                zbq = przq.tile([3 * 32 + 1, T], BF16, tag="zbq", bufs=2,
                                name=f"zbq{g0}")
                nc.scalar.activation(zbq[0:32 * (gn - 1) + 1, :],
                                     lnq[0:32 * (gn - 1) + 1, :],
                                     AF.Exp, scale=-1.0)
                for hh in range(g0, g0 + gn):
                    rb = prb.tile([DH, T], BF16, tag="rb", name=f"rb{hh}")
                    bcast(rb[:], zbq[32 * (hh - g0):32 * (hh - g0) + 1, :],
                          DH, T, "rc")
                    gg, pp = divmod(hh, 2)
                    src = oraws.pop(hh)[0:DH, :]
                    # PSUM sources (last pair) must go through DVE
                    eng = nc.vector if (hh % 2 == 0 or hh >= NH - 2) \
                        else nc.gpsimd
                    eng.tensor_mul(Oh[gg][pp * DH:(pp + 1) * DH, :],
                                   src, rb[:])
    bstk.close()
    wqstk.close()

    # wo projection + n2 LN (wo weights already resident via wo_pre)
    with tc.tile_pool(name="lnzB", bufs=8) as lnzB, \
         tc.tile_pool(name="lnauxB", bufs=1) as lnauxB, \
         tc.tile_pool(name="bcB2", bufs=2) as pbcB2, \
         tc.tile_pool(name="ppB3", bufs=1, space="PSUM") as ppB3, \
         tc.tile_pool(name="ppstB", bufs=2, space="PSUM") as ppstB:
        z3 = []

        def ep_wo(m, ps):
            zm = lnzB.tile([P, T], BF16, tag="z", name=f"z3_{m}")
            if fl["a_const_zero"]:
                nc.vector.tensor_add(zm[:], ps[:], x1[m][:])
            else:
                nc.vector.scalar_tensor_tensor(out=zm[:], in0=ps[:],
                                               scalar=vcol("a_const", m),
                                               in1=x1[m][:], op0=OP.add, op1=OP.add)
            z3.append(zm)

        proj("a_woT", Oh, ep_wo, wpo, ppB3, "wo", mh=2, ksplit=6,
             eng=nc.gpsimd, pre=wo_pre)
        x2 = layer_norm(z3, "n2", (ppstB, lnauxB, pbcB2),
                        lambda k: rtile(k, f"x2_{k}"))
    wostk.close()

    vr_proj([2], "b")  # covers the n2 LN serial chain
    pc_stk.close()  # chat tiles free

    # prefetch retriever-Q weights (in flight during the n2 LN tail)
    rwqstk = ExitStack()
    rwq = rwqstk.enter_context(tc.tile_pool(name="rwq", bufs=1))
    rwq_pre = preload_w(rwq, "r_wqT", "rwq", eng=nc.sync)

    # =========================================================================
    # Stage C: retriever attention + gated merge
    # =========================================================================
    with tc.tile_pool(name="ppC3", bufs=2, space="PSUM") as ppC3:
        Qr = []

        def ep_qr(m, ps):
            qm = pQr.tile([P, T], BF16, tag=f"qr{m}", name=f"qr{m}")
            if not fl["r_bq_zero"]:
                nc.vector.tensor_scalar_add(qm[:], ps[:], vcol("r_bq", m))
            elif m % 2 == 0:
                nc.scalar.activation(qm[:], ps[:], AF.Copy)
            else:
                nc.vector.tensor_copy(out=qm[:], in_=ps[:])
            Qr.append(qm)

        proj("r_wqT", x2, ep_qr, None, ppC3, "rwq", pre=rwq_pre)
    rwqstk.close()

    Or = [rtile(m, f"orr{m}") for m in range(HT)]
    with tc.tile_pool(name="pexpR", bufs=2) as pexpR, \
         tc.tile_pool(name="pbcR", bufs=2) as pbcR, \
         tc.tile_pool(name="ptwR", bufs=2) as ptw, \
         tc.tile_pool(name="ppSCr", bufs=2, space="PSUM") as ppSCr, \
         tc.tile_pool(name="ppsum", bufs=2, space="PSUM") as ppsum, \
         tc.tile_pool(name="ppAVr", bufs=1, space="PSUM") as ppAVr:
        for r in range(RH):
            for i in range(3):
                ets = []
                for kvt in range(2):
                    et = pexpR.tile([P, T], BF16, tag=f"re{kvt}",
                                    name=f"re{r}_{i}_{kvt}")
                    for c0 in range(0, T, NCH):
                        psS = ppSCr.tile([P, NCH], F32, tag="psc",
                                         name=f"rsc{r}{i}{kvt}{c0}")
                        for kc in range(2):
                            nc.tensor.matmul(
                                psS[:, :],
                                Kr[i][2 * r + kc][:, kvt * P:(kvt + 1) * P],
                                Qr[2 * r + kc][:, c0:c0 + NCH],
                                start=(kc == 0), stop=(kc == 1))
                        nc.scalar.activation(et[:, c0:c0 + NCH], psS[:],
                                             AF.Exp, scale=1.0 / 16.0)
                    ets.append(et)
                # Z via an all-ones [P,P] matmul: the partition-sum lands
                # broadcast on all 128 partitions, so 1/Z is a full-tile
                # Ln->Exp with no [1,T] rows and no DRAM broadcast.
                rbi = pbcR.tile([P, T], BF16, tag="rbr", name=f"rbr{r}_{i}")
                for c0 in range(0, T, NCH):
                    psZ = ppsum.tile([P, NCH], F32, tag="pz",
                                     name=f"rz{r}{i}{c0}")
                    for kvt in range(2):
                        nc.tensor.matmul(psZ[:, :], allones[:, :],
                                         ets[kvt][:, c0:c0 + NCH],
                                         start=(kvt == 0), stop=(kvt == 1))
                    lnz = pbcR.tile([P, NCH], F32, tag="lnz", bufs=2,
                                    name=f"lnz{r}{i}{c0}")
                    nc.scalar.activation(lnz[:], psZ[:], AF.Ln)
                    nc.scalar.activation(rbi[:, c0:c0 + NCH], lnz[:], AF.Exp,
                                         scale=-1.0)
                for md in range(2):
                    psA = ppAVr.tile([P, T], F32, tag=f"avr{md}",
                                     name=f"rav{r}{i}{md}")
                    col = RDH * r + P * md
                    for c0 in range(0, T, NCH):
                        for kvt in range(2):
                            nc.tensor.matmul(psA[:, c0:c0 + NCH],
                                             Vr[i][kvt][:, col:col + P],
                                             ets[kvt][:, c0:c0 + NCH],
                                             start=(kvt == 0), stop=(kvt == 1))
                    oT = ptw.tile([P, T], BF16, tag=f"ot{md}",
                                  name=f"ot{r}{i}{md}")
                    nc.vector.tensor_copy(out=oT[:], in_=psA[:])
                    dst = Or[2 * r + md]
                    eng_i = nc.vector if i == 2 else nc.gpsimd
                    if i == 0:
                        nc.gpsimd.tensor_mul(dst[:], oT[:], rbi[:])
                    else:
                        tw = ptw.tile([P, T], BF16, tag="tw", name=f"tw{r}{i}{md}")
                        eng_i.tensor_mul(tw[:], oT[:], rbi[:])
                        eng_i.tensor_add(dst[:], dst[:], tw[:])
    cstk2.close()

    # f_w2 preload (covers the merge/FFN span; lives through stage D)
    dstk = ExitStack()
    fw2 = dstk.enter_context(tc.tile_pool(name="fw2", bufs=1))
    fw2_tiles = []
    for k2 in range(4 * HT):
        wt = fw2.tile([P, H], BF16, tag=f"w2s{k2}", name=f"w2s{k2}")
        nc.scalar.dma_start(out=wt[:], in_=D["f_w2T"][k2 * P:(k2 + 1) * P, :])
        fw2_tiles.append(wt)

    x3 = []
    with tc.tile_pool(name="pcomb", bufs=1) as pcomb:
        with tc.tile_pool(name="rwo", bufs=1) as rwo, \
             tc.tile_pool(name="ppC5", bufs=2, space="PSUM") as ppC5:
            comb = []

            def ep_ro(m, ps):
                cm_ = pcomb.tile([P, T], BF16, tag=f"cb{m}", name=f"cb{m}")
                if fl["r_const_zero"] and m % 2 == 0:
                    nc.scalar.activation(cm_[:], ps[:], AF.Copy, bias=0.0,
                                         scale=1.0 / 3.0)
                elif fl["r_const_zero"]:
                    nc.vector.tensor_scalar_mul(cm_[:], ps[:], 1.0 / 3.0)
                else:
                    nc.vector.tensor_scalar(out=cm_[:], in0=ps[:], scalar1=1.0 / 3.0,
                                            scalar2=vcol("r_const", m),
                                            op0=OP.mult, op1=OP.add)
                comb.append(cm_)

            proj("r_woT", Or, ep_ro, rwo, ppC5, "rwo", mh=2, ksplit=6,
                 eng=nc.scalar)

        with tc.tile_pool(name="mgw", bufs=1) as mgw, \
             tc.tile_pool(name="pgw", bufs=2) as pgw, \
             tc.tile_pool(name="ptmp", bufs=2) as ptmp, \
             tc.tile_pool(name="ppC6", bufs=3, space="PSUM") as ppC6:
            for half in range(2):
                wb = load_wblocks(mgw, D["mg_wT"], 2 * HT, 4 * P, "mg",
                                  c0=half * 4 * P, bufs=2, eng=nc.scalar)
                for ml in range(4):
                    m = half * 4 + ml
                    ps = ppC6.tile([P, T], F32, tag="pbig", name=f"mgps{m}")
                    steps = [(wb[k][:, ml * P:(ml + 1) * P], x2[k][:])
                             for k in range(HT)]
                    steps += [(wb[HT + k][:, ml * P:(ml + 1) * P], comb[k][:])
                              for k in range(HT)]
                    mm(ps, steps)
                    gw = pgw.tile([P, T], BF16, tag="gw", name=f"gw{m}")
                    nc.scalar.activation(gw[:], ps[:], AF.Sigmoid,
                                         bias=vcol("mg_b", m))
                    eng1 = nc.vector if (m % 8 < 5) else nc.gpsimd
                    eng2 = nc.gpsimd if (m % 8 < 5) else nc.vector
                    d = ptmp.tile([P, T], BF16, tag="d", name=f"d{m}")
                    eng1.tensor_sub(d[:], x2[m][:], comb[m][:])
                    eng1.tensor_mul(d[:], gw[:], d[:])
                    s = ptmp.tile([P, T], BF16, tag="s", name=f"s{m}")
                    eng2.tensor_add(s[:], x2[m][:], comb[m][:])
                    xm = rtile(m, f"x3_{m}")
                    eng1.tensor_add(xm[:], s[:], d[:])
                    x3.append(xm)

    # =========================================================================
    # Stage D: FFN token-half-major + final LN + transpose to [T, H]
    # =========================================================================
    TH = T // 2
    with tc.tile_pool(name="pd", bufs=1) as pd, \
         tc.tile_pool(name="pdh", bufs=1) as pdh, \
         tc.tile_pool(name="pdz", bufs=1) as pdz, \
         tc.tile_pool(name="ppD1", bufs=2, space="PSUM") as ppD1, \
         tc.tile_pool(name="ppD2", bufs=1, space="PSUM") as ppD2, \
         tc.tile_pool(name="ppstD", bufs=2, space="PSUM") as ppstD, \
         tc.tile_pool(name="ppTD", bufs=1, space="PSUM") as ppT:
        for th in range(2):
            c0 = th * TH
            hts = []
            for mg_i in range(8):
                wblk = load_wblocks(pd, D["f_w1T"], HT, 512, "w1s",
                                    c0=mg_i * 512, bufs=2, eng=nc.sync)
                for ml in range(4):
                    m_abs = mg_i * 4 + ml
                    ps = ppD1.tile([P, TH], F32, tag="p1",
                                   name=f"f1ps{th}_{m_abs}")
                    mm(ps, [(wblk[k][:, ml * P:(ml + 1) * P],
                             x3[k][:, c0:c0 + TH]) for k in range(HT)])
                    htile = pdh.tile([P, TH], BF16, tag=f"h{m_abs}",
                                    name=f"h{th}_{m_abs}")
                    nc.scalar.activation(htile[:], ps[:], AF.Gelu,
                                         bias=vcol("f_b1", m_abs))
                    hts.append(htile)
            z4 = [None] * HT
            for grp in range(4):
                pso = [ppD2.tile([P, TH], F32, tag=f"g{j}",
                                 name=f"pso{th}_{grp}_{j}") for j in range(2)]
                for k2 in range(4 * HT):
                    wt = fw2_tiles[k2]
                    for j in range(2):
                        mo = grp * 2 + j
                        nc.tensor.matmul(pso[j][:, :],
                                         wt[:, mo * P:(mo + 1) * P],
                                         hts[k2][:],
                                         start=(k2 == 0), stop=(k2 == 4 * HT - 1))
                for j in range(2):
                    mo = grp * 2 + j
                    zm = pdz.tile([P, TH], BF16, tag="z4", bufs=8,
                                 name=f"z4_{th}_{mo}")
                    if fl["f_b2_zero"]:
                        nc.vector.tensor_add(zm[:], pso[j][:],
                                             x3[mo][:, c0:c0 + TH])
                    else:
                        nc.vector.scalar_tensor_tensor(
                            out=zm[:], in0=pso[j][:], scalar=vcol("f_b2", mo),
                            in1=x3[mo][:, c0:c0 + TH], op0=OP.add, op1=OP.add)
                    z4[mo] = zm
            with tc.tile_pool(name="lnauxD", bufs=1) as lnauxD, \
                 tc.tile_pool(name="bcD", bufs=2) as pbcD, \
                 tc.tile_pool(name="lnoD", bufs=8) as lnoD:
                fin = layer_norm(z4, "n3", (ppstD, lnauxD, pbcD),
                                 lambda k: lnoD.tile([P, TH], BF16, tag="fin",
                                                     name=f"fin{th}_{k}"),
                                 Tn=TH, nch=256)
                for tt in range(TH // P):
                    stg = pdz.tile([P, H], F32, tag="stg", bufs=2,
                                  name=f"stg{th}_{tt}")
                    for hh in range(2):
                        psT4 = ppT.tile([P, 512], BF16, tag="pt",
                                        name=f"pT{th}_{tt}_{hh}")
                        for j in range(4):
                            k2 = hh * 4 + j
                            nc.tensor.transpose(psT4[:, j * P:(j + 1) * P],
                                                fin[k2][:, tt * P:(tt + 1) * P],
                                                identb[:])
                        nc.vector.tensor_copy(
                            out=stg[:, hh * 512:(hh + 1) * 512], in_=psT4[:])
                    row0 = c0 + tt * P
                    nc.sync.dma_start(out=out_d[row0:row0 + P, :], in_=stg[:])
    dstk.close()
    ctx.close()


# =============================================================================
# Host side
# =============================================================================
_CACHE = {}


def _flags(g):
    def zero(a):
        return bool(np.all(a == 0.0))

    fl = {}
    for n in ("sln", "n1", "n2", "n3"):
        fl[f"{n}_trivial"] = bool(np.all(g[f"{n}_g"] == 1.0) and zero(g[f"{n}_b"]))
    fl["outp_b_zero"] = zero(g["outp_b"])
    wq_b, wk_b, wv_b = np.split(g["attn_in_b"], 3, 0)
    fl["a_bq_zero"] = zero(wq_b)
    a_const = wv_b @ g["attn_out_w"].T + g["attn_out_b"]
    fl["a_const_zero"] = zero(a_const)
    rq_b, rk_b, rv_b = np.split(g["retr_in_b"], 3, 0)
    fl["r_bq_zero"] = zero(rq_b)
    r_const = rv_b @ g["retr_out_w"].T + g["retr_out_b"]
    fl["r_const_zero"] = zero(r_const)
    for i in range(3):
        fl[f"c{i}_b2_zero"] = zero(g[f"c{i}_b2"])
    fl["f_b2_zero"] = zero(g["ffn_b2"])
    return fl, a_const, r_const


def _in_maps(g, a_const, r_const):
    import ml_dtypes
    bf16 = ml_dtypes.bfloat16

    def trb(a):
        return np.ascontiguousarray(np.ascontiguousarray(a.T).astype(bf16))

    wq, wk, wv = np.split(g["attn_in_w"], 3, 0)
    rq, rk, rv = np.split(g["retr_in_w"], 3, 0)

    vpack = np.zeros((P, VCOLS), np.float32)

    def setv(nm, vec):
        w = vec.shape[0] // P
        vpack[:, VOFF[nm]:VOFF[nm] + w] = vec.reshape(w, P).T

    setv("A", np.exp(g["A_log"]))
    setv("sg_b", g["sgate_b"])
    setv("Dp1", g["D"] + 1.0)
    setv("outp_b", g["outp_b"])
    setv("a_bq", np.split(g["attn_in_b"], 3, 0)[0])
    setv("a_const", a_const)
    setv("r_bq", np.split(g["retr_in_b"], 3, 0)[0])
    setv("r_const", r_const)
    setv("mg_b", g["mg_b"])
    setv("f_b1", g["ffn_b1"])
    setv("f_b2", g["ffn_b2"])
    for n in ("sln", "n1", "n2", "n3"):
        setv(f"{n}_g", g[f"{n}_g"])
        setv(f"{n}_b", g[f"{n}_b"])
    for i in range(3):
        setv(f"c{i}_b1", g[f"c{i}_b1"])
        setv(f"c{i}_b2", g[f"c{i}_b2"])

    shared = {
        "vpack": vpack,
        "wsgT": trb(g["sgate_w"]), "wBT": trb(g["B_w"]), "wCT": trb(g["C_w"]),
        "outp_wT": trb(g["outp_w"]),
        "a_wqT": trb(wq), "a_wkT": trb(wk), "a_wvT": trb(wv),
        "a_woT": trb(g["attn_out_w"]),
        "r_wqT": trb(rq), "r_wkT": trb(rk), "r_wvT": trb(rv),
        "r_woT": trb(g["retr_out_w"]),
        "mg_wT": trb(g["mg_w"]),
        "f_w1T": trb(g["ffn_w1"]), "f_w2T": trb(g["ffn_w2"]),
    }
    for i in range(3):
        shared[f"c{i}_w1T"] = trb(g[f"c{i}_w1"])
        shared[f"c{i}_w2T"] = trb(g[f"c{i}_w2"])

    in_maps = []
    for b in range(B):
        m = dict(shared)
        m["xT"] = trb(g["x"][b])
        for i in range(3):
            m[f"m{i}T"] = trb(g[f"mem{i}"][b, -256:, :])
        in_maps.append(m)
    return in_maps


def kernel(**inputs):
    g = {k: np.ascontiguousarray(np.asarray(v, dtype=np.float32))
         for k, v in inputs.items()}
    fl, a_const, r_const = _flags(g)

    key = tuple(sorted(fl.items()))
    if key not in _CACHE:
        _CACHE[key] = build_nc(fl)
    nc = _CACHE[key]

    in_maps = _in_maps(g, a_const, r_const)
    trace = os.environ.get("KERNEL_TRACE", "0") == "1"
    res = bass_utils.run_bass_kernel_spmd(nc, in_maps, core_ids=list(range(B)),
                                          trace=trace)
    global LAST_RESULTS
    LAST_RESULTS = res
    out = np.stack([res.results[b]["out"] for b in range(B)], axis=0)
    return out


LAST_RESULTS = None



# revision 52
# speedup vs baseline: 1.1654x; 1.1654x over previous
"""TRN2 Bass/Tile kernel for nn_DHSMBlock (SSM + self-attn + hierarchical memory + FFN).

Sharding: data-parallel over batch. B=8 rows -> 8 NeuronCores, one row per core,
no collectives. Each core gets the full weight set (host pre-transposed, bf16).

On-device layout is feature-major: every activation lives as X^T [feature, token]
so all matmuls contract over the partition dim. All matmul operands are bf16
(fp32 PSUM accumulation); fp32 is kept only for the SSM scan and LN statistics
rows. LayerNorm stats are ones-vector matmuls on the PE; rstd comes from an
ACT-engine Rsqrt; [1,T] -> [P,T] broadcasts go through a DRAM round-trip.
Self-attention score matmuls are row-packed two-heads-at-a-time (K=64 each) so
they run concurrently in disjoint PE row groups. The hierarchical-memory
compressors only depend on mem inputs, so they are emitted interleaved with the
SSM stage to cover its serial sections. Weight DMAs are spread over the SP/ACT
HW-DGE queues and the Pool SW-DGE queue, ordered by first use.
"""

import os
from contextlib import ExitStack

import numpy as np

os.environ.setdefault("MYCRO_LOCAL_CACHE", "1")

import concourse.bass as bass
import concourse.mybir as mybir
import concourse.tile as tile
from concourse import bass_utils
from concourse.masks import make_identity

F32 = mybir.dt.float32
BF16 = mybir.dt.bfloat16
AF = mybir.ActivationFunctionType
OP = mybir.AluOpType

B, T, H, S = 8, 1024, 1024, 128
NH, DH = 16, 64          # self-attention heads
RH, RDH = 4, 256         # retriever heads
COMP = [1024, 512, 256]  # compressor widths
P = 128
HT = H // P              # 8 feature tiles
NCH = 512                # matmul moving-dim chunk (one fp32 PSUM bank)
EPS = 1e-5

# packed per-partition vectors: (name, rows); layout (k p) -> [p, k]
VDEFS = [
    ("A", S), ("sg_b", S), ("Dp1", H), ("outp_b", H), ("a_bq", H), ("a_const", H),
    ("r_bq", H), ("r_const", H), ("mg_b", H), ("f_b1", 4 * H), ("f_b2", H),
    ("sln_g", H), ("sln_b", H), ("n1_g", H), ("n1_b", H),
    ("n2_g", H), ("n2_b", H), ("n3_g", H), ("n3_b", H),
    ("c0_b1", 1024), ("c1_b1", 512), ("c2_b1", 256),
    ("c0_b2", H), ("c1_b2", H), ("c2_b2", H),
]
VOFF = {}
_o = 0
for _n, _r in VDEFS:
    VOFF[_n] = _o
    _o += _r // P
VCOLS = _o


def build_nc(fl):
    nc = bass.Bass("TRN2", target_bir_lowering=False, debug=False, num_devices=8)
    D = {}

    def din(name, shape, dt=BF16):
        D[name] = nc.dram_tensor(name, list(shape), dt, kind="ExternalInput").ap()

    din("xT", (H, T))
    for i in range(3):
        din(f"m{i}T", (H, 256))
    din("wsgT", (H, S)); din("wBT", (H, S)); din("wCT", (S, H))
    din("vpack", (P, VCOLS), F32)
    din("outp_wT", (H, H))
    din("a_wqT", (H, H)); din("a_wkT", (H, H)); din("a_wvT", (H, H))
    din("a_woT", (H, H))
    for i, c in enumerate(COMP):
        din(f"c{i}_w1T", (H, c)); din(f"c{i}_w2T", (c, H))
    din("r_wqT", (H, H)); din("r_wkT", (H, H)); din("r_wvT", (H, H))
    din("r_woT", (H, H))
    din("mg_wT", (2 * H, H))
    din("f_w1T", (H, 4 * H)); din("f_w2T", (4 * H, H))
    out_d = nc.dram_tensor("out", [T, H], F32, kind="ExternalOutput").ap()

    with tile.TileContext(nc, pool_alloc_mode="queue") as tc:
        _body(nc, tc, D, out_d, fl)
    _split_matmul_waits(nc)
    return nc


_WAIT_EXEMPT = {
    "InstEventSemaphore", "InstAllEngineBarrier",
    "InstUnconditionalBranch", "InstCompareAndBranch", "InstIndirectBranch",
    "InstHalt", "InstBranchHint",
}


def _split_matmul_waits(nc):
    """TPB engine instruction encodings carry at most one sync wait; move
    surplus waits onto a preceding same-engine no-op (sequencer WAITs)."""
    import bass_rust
    cnt = 0
    for f in nc.m.functions:
        for blk in f.blocks:
            insts = blk.instructions
            out = []
            changed = False
            for inst in insts:
                if (type(inst).__name__ not in _WAIT_EXEMPT
                        and not isinstance(inst, bass_rust.InstISA)):
                    si = inst.sync_info
                    if si is not None and len(si.on_wait) > 1:
                        surplus = list(si.on_wait[:-1])
                        for j in range(0, len(surplus), 2):
                            ev = bass_rust.InstEventSemaphore(name=f"I-wsplit-{cnt}")
                            cnt += 1
                            ev.engine = inst.engine
                            ev.bass_nofuse = True
                            ev.sync_info = bass_rust.SyncInfo(
                                on_wait=surplus[j:j + 2], on_update=[])
                            out.append(ev)
                        inst.sync_info = bass_rust.SyncInfo(
                            on_wait=[si.on_wait[-1]], on_update=list(si.on_update))
                        changed = True
                out.append(inst)
            if changed:
                blk.instructions = out
    return nc


def _body(nc, tc, D, out_d, fl):
    import itertools
    _bc_ctr = itertools.count()
    ctx = ExitStack()

    # ---------- ambient pools ----------
    pv = ctx.enter_context(tc.tile_pool(name="pv", bufs=1))
    resid = ctx.enter_context(tc.tile_pool(name="resid", bufs=2))
    dscr = ctx.enter_context(tc.tile_pool(name="dscr", bufs=4, space="DRAM"))
    # long-lived mid-stage pools, entered in reverse close order:
    # pKr/pVr/pQr close at the end of the retriever attention; pc (chat)
    # closes after the Kr/Vr projections.
    cstk2 = ExitStack()
    pKr = cstk2.enter_context(tc.tile_pool(name="pKr", bufs=1))
    pVr = cstk2.enter_context(tc.tile_pool(name="pVr", bufs=1))
    pQr = cstk2.enter_context(tc.tile_pool(name="pQr", bufs=1))
    pc_stk = ExitStack()
    pc = pc_stk.enter_context(tc.tile_pool(name="pc", bufs=1))

    def bcast(dst_ap, src_ap, parts, tn, tag, dt=BF16):
        """Broadcast a [1,tn] SBUF row to [parts,tn] via a DRAM round-trip."""
        scr = dscr.tile([1, tn], dt, tag=tag, name=f"scr_{tag}_{next(_bc_ctr)}")
        nc.sync.dma_start(out=scr[:], in_=src_ap)
        nc.sync.dma_start(out=dst_ap, in_=scr[0:1, :].broadcast_to((parts, tn)))

    def rtile(k, name):
        return resid.tile([P, T], BF16, tag=f"r{k}", name=name)

    # packed vectors (one DMA)
    vpk = pv.tile([P, VCOLS], F32, tag="vpk")
    nc.scalar.dma_start(out=vpk[:], in_=D["vpack"][:, :])

    def vcol(nm, k=0):
        o = VOFF[nm] + k
        return vpk[:, o:o + 1]

    # all-ones [P, P] stationary: partition-sum matmuls whose outputs land
    # broadcast on all 128 partitions AND that count as full-array activity
    # for the PE HAM clock gate (M=1 ones-column matmuls read as idle and
    # keep the PE throttled at 1.2 GHz).
    allones = pv.tile([P, P], BF16, tag="allones")
    nc.vector.memset(allones[:], 1.0)
    eps_t = pv.tile([P, 1], F32, tag="eps")
    nc.vector.memset(eps_t[:], EPS)
    identb = pv.tile([P, P], BF16, tag="identb")
    make_identity(nc, identb[:])

    xs = []
    for k in range(HT):
        t = rtile(k, f"x_{k}")
        nc.sync.dma_start(out=t[:], in_=D["xT"][k * P:(k + 1) * P, :])
        xs.append(t)

    # ---------- helpers ----------
    def mm(ps, steps, nch=NCH):
        """ps[M,N] = sum_k steps[k].lhsT.T @ steps[k].rhs ; chunks the moving dim."""
        n = ps.shape[-1]
        K = len(steps)
        for c0 in range(0, n, nch):
            ce = min(c0 + nch, n)
            for k, (lt, rt) in enumerate(steps):
                nc.tensor.matmul(ps[:, c0:ce], lt, rt[:, c0:ce],
                                 start=(k == 0), stop=(k == K - 1))

    def load_wblocks(pool, dram_ap, nk, cols, tag, c0=0, bufs=1, eng=None,
                     tile_cols=None):
        eng = eng or nc.scalar
        tiles = []
        for k in range(nk):
            t = pool.tile([P, tile_cols or cols], BF16, tag=f"{tag}{k}", bufs=bufs,
                          name=f"{tag}{k}_{c0}")
            eng.dma_start(out=t[:, 0:cols],
                          in_=dram_ap[k * P:(k + 1) * P, c0:c0 + cols])
            tiles.append(t)
        return tiles

    def preload_w(pool, wname, tag, nk=HT, mh=4, eng=None, halves=(0, 1)):
        """Early weight loads for a later proj(): emitting the DMAs from a
        pool created before the current stage's pools keeps them out of the
        SBUF ring's wait-for-free chain at the stage boundary."""
        return {half: load_wblocks(pool, D[wname], nk, mh * P, tag,
                                   c0=half * mh * P, bufs=1, eng=eng)
                for half in halves}

    def proj(wname, rhs_tiles, epilogue, pool, ppool, tag, nk=HT, mh=4, wbufs=2,
             eng=None, pre=None, ksplit=None):
        """out[m] = epilogue(m, psum(W^T[:,m] @ rhs)), streaming W in col-halves.

        ksplit=j: accumulate k<j for every m of the group first, then finish
        k>=j -- covers late-arriving rhs tiles (attention epilogue tails)
        with ~mh*j*2 matmuls of PE work instead of stalling."""
        for half in range(HT // mh):
            if pre is not None and half in pre:
                wb = pre[half]
            else:
                wb = load_wblocks(pool, D[wname], nk, mh * P, tag,
                                  c0=half * mh * P, bufs=wbufs, eng=eng)
            if ksplit is None:
                for ml in range(mh):
                    m = half * mh + ml
                    ps = ppool.tile([P, T], F32, tag="pbig", name=f"{tag}ps{m}")
                    mm(ps, [(wb[k][:, ml * P:(ml + 1) * P], rhs_tiles[k][:])
                            for k in range(nk)])
                    epilogue(m, ps)
            else:
                pss = []
                for ml in range(mh):
                    m = half * mh + ml
                    ps = ppool.tile([P, T], F32, tag=f"pb{ml}",
                                    name=f"{tag}ps{m}")
                    for c0 in range(0, T, NCH):
                        for k in range(ksplit):
                            nc.tensor.matmul(ps[:, c0:c0 + NCH],
                                             wb[k][:, ml * P:(ml + 1) * P],
                                             rhs_tiles[k][:, c0:c0 + NCH],
                                             start=(k == 0), stop=False)
                    pss.append(ps)
                for ml in range(mh):
                    m = half * mh + ml
                    ps = pss[ml]
                    for c0 in range(0, T, NCH):
                        for k in range(ksplit, nk):
                            nc.tensor.matmul(ps[:, c0:c0 + NCH],
                                             wb[k][:, ml * P:(ml + 1) * P],
                                             rhs_tiles[k][:, c0:c0 + NCH],
                                             start=False, stop=(k == nk - 1))
                    epilogue(m, ps)

    def layer_norm(z, gname, pools, mk_out, Tn=T, nch=NCH):
        """Feature-dim (partition) LN over bf16 z tiles. Stats come from
        all-ones [P,P] matmuls, so mean/var land already broadcast on all
        partitions -- no [1,T] row ops and no DRAM broadcast round-trip.
        mk_out(k) -> bf16 tile."""
        pp_stat, lnaux, _ = pools
        nchunk = max(1, Tn // nch)
        cw = min(Tn, nch)
        u = next(_bc_ctr)
        outs = [mk_out(k) for k in range(HT)]
        for c in range(nchunk):
            cs = slice(c * cw, (c + 1) * cw)
            ps_s = pp_stat.tile([P, cw], F32, tag="st",
                                name=f"lnps_s{c}_{u}")
            ps_q = pp_stat.tile([P, cw], F32, tag="st",
                                name=f"lnps_q{c}_{u}")
            for k in range(HT):
                nc.tensor.matmul(ps_s[:, :], allones[:, :], z[k][:, cs],
                                 start=(k == 0), stop=(k == HT - 1))
            for k in range(HT):
                sq = lnaux.tile([P, cw], BF16, tag="lnsq", bufs=3)
                # z^2 on ACT (Square is in every table; ACT is the least
                # loaded engine at every LN site)
                if k % 4 == 3:
                    nc.gpsimd.tensor_mul(sq[:], z[k][:, cs], z[k][:, cs])
                else:
                    nc.scalar.activation(sq[:], z[k][:, cs], AF.Square)
                nc.tensor.matmul(ps_q[:, :], allones[:, :], sq[:, :],
                                 start=(k == 0), stop=(k == HT - 1))
            mean_c = lnaux.tile([P, cw], F32, tag="mean", bufs=1,
                                name=f"ln_mean{c}_{u}")
            nc.scalar.activation(mean_c[:], ps_s[:], AF.Copy, bias=0.0,
                                 scale=1.0 / H)
            m2 = lnaux.tile([P, cw], F32, tag="m2", bufs=1, name=f"ln_m2{c}_{u}")
            nc.vector.tensor_mul(m2[:], mean_c[:], mean_c[:])
            var_c = lnaux.tile([P, cw], F32, tag="var", bufs=1,
                               name=f"ln_var{c}_{u}")
            nc.vector.scalar_tensor_tensor(out=var_c[:], in0=ps_q[:],
                                           scalar=1.0 / H, in1=m2[:],
                                           op0=OP.mult, op1=OP.subtract)
            # rstd = exp(-0.5*ln(var+eps)): stays in the Ln/Exp act table
            # used by every softmax in the kernel -- no act-table switches.
            lnv = lnaux.tile([P, cw], F32, tag="lnv", bufs=1,
                             name=f"ln_lnv{c}_{u}")
            nc.scalar.activation(lnv[:], var_c[:], AF.Ln, bias=eps_t[:, 0:1])
            rstd_c = lnaux.tile([P, cw], BF16, tag="rstd", bufs=2,
                                name=f"ln_rstd{c}_{u}")
            nc.scalar.activation(rstd_c[:], lnv[:], AF.Exp, scale=-0.5)
            mr_c = lnaux.tile([P, cw], BF16, tag="mr", bufs=2,
                              name=f"ln_mr{c}_{u}")
            nc.vector.tensor_mul(mr_c[:], mean_c[:], rstd_c[:])
            for k in range(HT):
                o = outs[k]
                # weighted split: DVE ~1.6x faster than Pool per op
                eng = nc.vector if k < 6 else nc.gpsimd
                eng.tensor_mul(o[:, cs], z[k][:, cs], rstd_c[:])
                eng.tensor_sub(o[:, cs], o[:, cs], mr_c[:])
                if not fl[f"{gname}_trivial"]:
                    nc.vector.tensor_scalar(out=o[:, cs], in0=o[:, cs],
                                            scalar1=vcol(f"{gname}_g", k),
                                            scalar2=vcol(f"{gname}_b", k),
                                            op0=OP.mult, op1=OP.add)
        return outs

    # =========================================================================
    # Stage A: SSM layer, emission interleaved with the hierarchical-memory
    # compressors (which only depend on mem inputs) to cover serial sections.
    # =========================================================================
    ssm = ExitStack()
    ssm1 = ssm.enter_context(tc.tile_pool(name="ssm1", bufs=1))
    comp = ssm.enter_context(tc.tile_pool(name="comp", bufs=1))
    ppc = ssm.enter_context(tc.tile_pool(name="ppc", bufs=2, space="PSUM"))
    lnz = ssm.enter_context(tc.tile_pool(name="lnzA", bufs=8))
    lnaux = ssm.enter_context(tc.tile_pool(name="lnauxA", bufs=1))
    pbc = ssm.enter_context(tc.tile_pool(name="bcA", bufs=1))
    lnout = ssm.enter_context(tc.tile_pool(name="lnoA", bufs=8))
    ppst = ssm.enter_context(tc.tile_pool(name="ppstA", bufs=2, space="PSUM"))

    wsg = load_wblocks(ssm1, D["wsgT"], HT, S, "wsg", eng=nc.gpsimd)
    wB = load_wblocks(ssm1, D["wBT"], HT, S, "wB", eng=nc.gpsimd)
    wC = ssm1.tile([S, H], BF16, tag="wC")
    nc.gpsimd.dma_start(out=wC[:], in_=D["wCT"][:, :])

    chat = {}
    _comp_mid = {}

    def comp_mid(i):
        cwid = COMP[i]
        ct = cwid // P
        mT = load_wblocks(comp, D[f"m{i}T"], HT, 256, "mT", bufs=1, eng=nc.sync)
        mid = []
        for h0 in range(0, ct, 4):
            hc = min(4, ct - h0)
            w1 = load_wblocks(comp, D[f"c{i}_w1T"], HT, hc * P, "cw1_", bufs=2,
                              c0=h0 * P, tile_cols=512, eng=nc.scalar)
            for cm in range(hc):
                ps = ppc.tile([P, 256], F32, tag="pc1", name=f"pm{i}_{h0 + cm}")
                mm(ps, [(w1[k][:, cm * P:(cm + 1) * P], mT[k][:])
                        for k in range(HT)])
                md = comp.tile([P, 256], BF16, tag=f"mid{h0 + cm}",
                               name=f"mid{i}_{h0 + cm}")
                nc.scalar.activation(md[:], ps[:], AF.Relu,
                                     bias=vcol(f"c{i}_b1", h0 + cm))
                mid.append(md)
        _comp_mid[i] = mid

    def comp_chat(i):
        mid = _comp_mid.pop(i)
        ct = COMP[i] // P
        chat[i] = []
        for h0 in range(2):
            w2 = load_wblocks(comp, D[f"c{i}_w2T"], ct, 512, "cw2_", bufs=2,
                              c0=h0 * 512, tile_cols=512, eng=nc.scalar)
            for ml in range(4):
                m = h0 * 4 + ml
                ps = ppc.tile([P, 256], F32, tag="pc1", name=f"pc{i}_{m}")
                mm(ps, [(w2[k][:, ml * P:(ml + 1) * P], mid[k][:])
                        for k in range(ct)])
                cm_t = pc.tile([P, 256], BF16, tag=f"c{i}_{m}", name=f"c{i}_{m}")
                if fl[f"c{i}_b2_zero"]:
                    nc.scalar.activation(cm_t[:], ps[:], AF.Copy)
                else:
                    nc.vector.tensor_scalar_add(cm_t[:], ps[:],
                                                vcol(f"c{i}_b2", m))
                chat[i].append(cm_t)

    def kr_proj(i):
        """Retriever K projection of compressed memory i (fills LN gaps)."""
        Kr[i] = []
        with tc.tile_pool(name=f"rkw{i}", bufs=1) as rkw:
            for hf in range(2):
                wk = load_wblocks(rkw, D["r_wkT"], HT, 512, "wkr",
                                  c0=hf * 512, bufs=1, eng=nc.gpsimd)
                for ml in range(4):
                    m = hf * 4 + ml
                    ps = ppc.tile([P, 256], F32, tag="pc1", name=f"pk{i}_{m}")
                    mm(ps, [(wk[k][:, ml * P:(ml + 1) * P], chat[i][k][:])
                            for k in range(HT)])
                    kt_ = pKr.tile([P, 256], BF16, tag=f"kr{i}_{m}",
                                   name=f"kr{i}_{m}")
                    nc.scalar.activation(kt_[:], ps[:], AF.Copy)
                    Kr[i].append(kt_)

    def vr_proj(idxs, tag):
        """Retriever V projection of compressed memories (fills LN gaps)."""
        with tc.tile_pool(name=f"rvw{tag}", bufs=1) as rvw, \
             tc.tile_pool(name=f"ppvr{tag}", bufs=2, space="PSUM") as ppvr:
            wvr = load_wblocks(rvw, D["r_wvT"], HT, H, "wvr", eng=nc.scalar)
            for i in idxs:
                Vr[i] = []
                for kvt in range(2):
                    ps = ppvr.tile([P, T], F32, tag="pkv2",
                                   name=f"pv{i}_{kvt}")
                    mm(ps, [(chat[i][k][:, kvt * P:(kvt + 1) * P], wvr[k][:])
                            for k in range(HT)])
                    vt = pVr.tile([P, T], BF16, tag=f"vr{i}_{kvt}",
                                  name=f"vr{i}_{kvt}")
                    nc.scalar.activation(vt[:], ps[:], AF.Copy)
                    Vr[i].append(vt)

    Kr, Vr = {}, {}
    z1 = []
    with tc.tile_pool(name="ppA", bufs=2, space="PSUM") as ppA:
        psG = ppA.tile([P, T], F32, tag="pbig")
        mm(psG, [(wsg[k][:], xs[k][:]) for k in range(HT)])
        gate = ssm1.tile([P, T], BF16, tag="gate")
        nc.scalar.activation(gate[:], psG[:], AF.Sigmoid, bias=vcol("sg_b"))
        psB = ppA.tile([P, T], F32, tag="pbig")
        mm(psB, [(wB[k][:], xs[k][:]) for k in range(HT)])
        u = ssm1.tile([P, T], F32, tag="u")
        nc.vector.tensor_mul(u[:], psB[:], gate[:])
        states = ssm1.tile([P, T], F32, tag="states")
        nc.vector.tensor_tensor_scan(states[:], vcol("A").to_broadcast((P, T)),
                                     u[:], 0.0, op0=OP.mult, op1=OP.add)
        states_bf = ssm1.tile([P, T], BF16, tag="states_bf")
        nc.vector.tensor_copy(out=states_bf[:], in_=states[:])

        comp_mid(0)  # covers the scan + z1 serial section

        for m in range(HT):
            psY = ppA.tile([P, T], F32, tag="pbig", name=f"psY{m}")
            mm(psY, [(wC[:, m * P:(m + 1) * P], states_bf[:])])
            zm = lnz.tile([P, T], BF16, tag="z", name=f"z1_{m}")
            nc.vector.scalar_tensor_tensor(out=zm[:], in0=xs[m][:],
                                           scalar=vcol("Dp1", m), in1=psY[:],
                                           op0=OP.mult, op1=OP.add)
            z1.append(zm)

    comp_chat(0)

    # prefetch outp weights so they are resident when the sln LN completes
    woustk = ExitStack()
    wou = woustk.enter_context(tc.tile_pool(name="wouA", bufs=1))
    wou_pre = preload_w(wou, "outp_wT", "wou", halves=(0,), eng=nc.gpsimd)

    ln1 = layer_norm(z1, "sln", (ppst, lnaux, pbc),
                     lambda k: lnout.tile([P, T], BF16, tag="ln1", name=f"ln1_{k}"))

    kr_proj(0)  # covers the sln LN serial chain
    comp_mid(1)

    z2 = []

    def ep_outp(m, ps):
        zm = lnz.tile([P, T], BF16, tag="z", name=f"z2_{m}")
        if fl["outp_b_zero"]:
            nc.vector.tensor_add(zm[:], ps[:], xs[m][:])
        else:
            nc.vector.scalar_tensor_tensor(out=zm[:], in0=ps[:],
                                           scalar=vcol("outp_b", m),
                                           in1=xs[m][:], op0=OP.add, op1=OP.add)
        z2.append(zm)

    with tc.tile_pool(name="ppA2", bufs=2, space="PSUM") as ppA2:
        proj("outp_wT", ln1, ep_outp, wou, ppA2, "wou", wbufs=1,
             eng=nc.gpsimd, pre=wou_pre)
    woustk.close()

    comp_chat(1)

    x1 = layer_norm(z2, "n1", (ppst, lnaux, pbc), lambda k: rtile(k, f"x1_{k}"))

    kr_proj(1)  # covers the n1 LN serial chain
    comp_mid(2)
    comp_chat(2)
    kr_proj(2)
    ssm.close()
    # prefetch pools, opened here so their ring slots precede the attention
    # pools (LIFO close order: wq0 closes after bstk, wpo after rwq).
    wostk = ExitStack()
    wpo = wostk.enter_context(tc.tile_pool(name="woB", bufs=1))
    wo_pre = preload_w(wpo, "a_woT", "wo", mh=2, halves=(0, 1, 2, 3),
                       eng=nc.gpsimd)
    wqstk = ExitStack()
    wq0 = wqstk.enter_context(tc.tile_pool(name="wq0", bufs=1))
    wq_pre = preload_w(wq0, "a_wqT", "wq0", halves=(0,), eng=nc.gpsimd)
    vr_proj([0, 1], "a")  # covers the n1->QKV transition

    # =========================================================================
    # Stage B: self-attention. One head at a time; every matmul uses the full
    # 128x128 PE array (zero-padded K rows / V columns) so the HAM clock gate
    # sees the PE as busy and keeps it at 2.4 GHz -- partial-array matmuls
    # (K=64 scores, M=65 AV) left the whole attention region throttled at
    # 1.2 GHz.
    # =========================================================================
    Oh = [rtile(g, f"oh{g}") for g in range(HT)]
    bstk = ExitStack()
    pQ = bstk.enter_context(tc.tile_pool(name="pQ", bufs=1))
    pK = bstk.enter_context(tc.tile_pool(name="pK", bufs=1))
    pV = bstk.enter_context(tc.tile_pool(name="pV", bufs=1))
    # Kz[h]: [P, T] with the other head's 64 feature rows zeroed -> K=128
    # score matmuls with a full Q rhs.
    Kz = [pK.tile([P, T], BF16, tag=f"kz{h}", name=f"kz{h}")
          for h in range(NH)]
    for h in range(NH):
        zs = slice(DH, P) if (h % 2 == 0) else slice(0, DH)
        (nc.vector if h % 2 == 0 else nc.gpsimd).memset(Kz[h][zs, :], 0.0)
    with tc.tile_pool(name="wqkv", bufs=1) as wqkv, \
         tc.tile_pool(name="ppB1", bufs=2, space="PSUM") as ppB1:
        Qh, Vp = [], []

        def ep_q(m, ps):
            qm = pQ.tile([P, T], BF16, tag=f"q{m}", name=f"q{m}")
            if fl["a_bq_zero"]:
                nc.scalar.activation(qm[:], ps[:], AF.Copy)
            else:
                nc.vector.tensor_scalar_add(qm[:], ps[:], vcol("a_bq", m))
            Qh.append(qm)

        def ep_k(m, ps):
            nc.vector.tensor_copy(out=Kz[2 * m][0:DH, :], in_=ps[0:DH, :])
            nc.scalar.activation(Kz[2 * m + 1][DH:P, :], ps[DH:P, :], AF.Copy)

        proj("a_wqT", x1, ep_q, wqkv, ppB1, "wq", wbufs=1, eng=nc.gpsimd,
             pre=wq_pre)
        proj("a_wkT", x1, ep_k, wqkv, ppB1, "wq", wbufs=1, eng=nc.gpsimd)
        # V token-major with a ones column appended per head (Z row)
        VW = DH + 1
        for kt in range(HT):
            vt = pV.tile([P, NH * VW], BF16, tag=f"v{kt}", name=f"v{kt}")
            vv = vt[:].rearrange("p (h c) -> p h c", c=VW)
            nc.gpsimd.memset(vv[:, :, DH:DH + 1], 1.0)
            Vp.append(vt)
        for vh in range(2):
            wvb = load_wblocks(wqkv, D["a_wvT"], HT, 512, "wq", c0=vh * 512,
                               eng=nc.gpsimd)
            for kt in range(HT):
                ps = ppB1.tile([P, 512], F32, tag="pvh", name=f"psV{vh}_{kt}")
                mm(ps, [(x1[k][:, kt * P:(kt + 1) * P], wvb[k][:])
                        for k in range(HT)])
                nc.vector.tensor_copy(
                    out=Vp[kt][:].rearrange("p (h c) -> p h c", c=VW)[
                        :, 8 * vh:8 * (vh + 1), 0:DH],
                    in_=ps[:].rearrange("p (h c) -> p h c", c=DH)[:, :, :])

    with tc.tile_pool(name="pexp", bufs=3) as pexp, \
         tc.tile_pool(name="poraw", bufs=5) as poraw, \
         tc.tile_pool(name="przq", bufs=1) as przq, \
         tc.tile_pool(name="prb", bufs=3) as prb, \
         tc.tile_pool(name="ppS", bufs=2, space="PSUM") as ppS, \
         tc.tile_pool(name="ppA", bufs=2, space="PSUM") as ppA:
        # Z-row groups: quads early, pairs at the end so the final 1/Z chain
        # (the wo-proj's gating input) is short; head 15's PSUM drains go to
        # ACT, which is idle after its last exp while DVE still has backlog.
        GRP = {}
        for h0, gn in ((0, 4), (4, 4), (8, 4), (12, 2), (14, 2)):
            for hh in range(h0, h0 + gn):
                GRP[hh] = (h0, gn)
        zq = None
        oraws = {}
        for h in range(NH):
            g, par = divmod(h, 2)
            psA = ppA.tile([DH + 1, T], F32, tag="pa", name=f"psA{h}")
            ets = {}
            # software pipeline: emit scores(kt), exp(kt), then AV(kt-1)
            for kt in range(HT + 1):
                if kt < HT:
                    psS = ppS.tile([P, T], F32, tag="ps", name=f"psS{h}_{kt}")
                    for c0 in range(0, T, NCH):
                        nc.tensor.matmul(psS[:, c0:c0 + NCH],
                                         Kz[h][:, kt * P:(kt + 1) * P],
                                         Qh[g][:, c0:c0 + NCH],
                                         start=True, stop=True)
                    et = pexp.tile([P, T], BF16, tag="et", name=f"et{h}_{kt}")
                    nc.scalar.activation(et[:], psS[:], AF.Exp, scale=1.0 / 8.0)
                    ets[kt] = et
                if kt > 0:
                    pkt = kt - 1
                    vs = Vp[pkt][:, h * VW:(h + 1) * VW]
                    for c0 in range(0, T, NCH):
                        nc.tensor.matmul(psA[:, c0:c0 + NCH], vs,
                                         ets[pkt][:, c0:c0 + NCH],
                                         start=(pkt == 0), stop=(pkt == HT - 1))
            # drain PSUM promptly: bf16 copy of AV rows + f32 copy of the Z
            # row into a quad-packed tile; 1/Z (Ln->Exp, same act table as
            # the softmax Exp) + DRAM-broadcast + apply happen from SBUF,
            # off the Tensor/PSUM critical path.
            # Z rows pack at 32-aligned partitions (engine outputs must start
            # at partition 0/32/64/96); the in-between rows are never read.
            g0, gn = GRP[h]
            if h == g0:
                zq = przq.tile([3 * 32 + 1, T], F32, tag="zq", bufs=1,
                               name=f"zq{g0}")
            zrow = 32 * (h - g0)
            nc.vector.tensor_copy(out=zq[zrow:zrow + 1, :],
                                  in_=psA[DH:DH + 1, :])
            if h >= NH - 2:
                # last pair: no successor reuses these PSUM banks, so apply
                # straight from PSUM and skip the bf16 drain copy
                oraws[h] = psA
            else:
                oraw = poraw.tile([DH, T], BF16, tag="or", name=f"oraw{h}")
                nc.vector.tensor_copy(out=oraw[:], in_=psA[0:DH, :])
                oraws[h] = oraw
            if h == g0 + gn - 1:
                lnq = przq.tile([3 * 32 + 1, T], F32, tag="lnq", bufs=1,
                                name=f"lnq{g0}")
                nc.scalar.activation(lnq[0:32 * (gn - 1) + 1, :],
                                     zq[0:32 * (gn - 1) + 1, :], AF.Ln)
                zbq = przq.tile([3 * 32 + 1, T], BF16, tag="zbq", bufs=2,
                                name=f"zbq{g0}")
                nc.scalar.activation(zbq[0:32 * (gn - 1) + 1, :],
                                     lnq[0:32 * (gn - 1) + 1, :],
                                     AF.Exp, scale=-1.0)
                for hh in range(g0, g0 + gn):
                    rb = prb.tile([DH, T], BF16, tag="rb", name=f"rb{hh}")
                    bcast(rb[:], zbq[32 * (hh - g0):32 * (hh - g0) + 1, :],
                          DH, T, "rc")
                    gg, pp = divmod(hh, 2)
                    src = oraws.pop(hh)[0:DH, :]
                    # PSUM sources (last pair) must go through DVE
                    eng = nc.vector if (hh % 2 == 0 or hh >= NH - 2) \
                        else nc.gpsimd
                    eng.tensor_mul(Oh[gg][pp * DH:(pp + 1) * DH, :],
                                   src, rb[:])
    bstk.close()
    wqstk.close()

    # wo projection + n2 LN (wo weights already resident via wo_pre)
    with tc.tile_pool(name="lnzB", bufs=8) as lnzB, \
         tc.tile_pool(name="lnauxB", bufs=1) as lnauxB, \
         tc.tile_pool(name="bcB2", bufs=2) as pbcB2, \
         tc.tile_pool(name="ppB3", bufs=1, space="PSUM") as ppB3, \
         tc.tile_pool(name="ppstB", bufs=2, space="PSUM") as ppstB:
        z3 = []

        def ep_wo(m, ps):
            zm = lnzB.tile([P, T], BF16, tag="z", name=f"z3_{m}")
            if fl["a_const_zero"]:
                nc.vector.tensor_add(zm[:], ps[:], x1[m][:])
            else:
                nc.vector.scalar_tensor_tensor(out=zm[:], in0=ps[:],
                                               scalar=vcol("a_const", m),
                                               in1=x1[m][:], op0=OP.add, op1=OP.add)
            z3.append(zm)

        proj("a_woT", Oh, ep_wo, wpo, ppB3, "wo", mh=2, ksplit=6,
             eng=nc.gpsimd, pre=wo_pre)
        x2 = layer_norm(z3, "n2", (ppstB, lnauxB, pbcB2),
                        lambda k: rtile(k, f"x2_{k}"))
    wostk.close()

    vr_proj([2], "b")  # covers the n2 LN serial chain
    pc_stk.close()  # chat tiles free

    # prefetch retriever-Q weights (in flight during the n2 LN tail)
    rwqstk = ExitStack()
    rwq = rwqstk.enter_context(tc.tile_pool(name="rwq", bufs=1))
    rwq_pre = preload_w(rwq, "r_wqT", "rwq", eng=nc.gpsimd)

    # =========================================================================
    # Stage C: retriever attention + gated merge
    # =========================================================================
    with tc.tile_pool(name="ppC3", bufs=2, space="PSUM") as ppC3:
        Qr = []

        def ep_qr(m, ps):
            qm = pQr.tile([P, T], BF16, tag=f"qr{m}", name=f"qr{m}")
            if not fl["r_bq_zero"]:
                nc.vector.tensor_scalar_add(qm[:], ps[:], vcol("r_bq", m))
            elif m % 2 == 0:
                nc.scalar.activation(qm[:], ps[:], AF.Copy)
            else:
                nc.vector.tensor_copy(out=qm[:], in_=ps[:])
            Qr.append(qm)

        proj("r_wqT", x2, ep_qr, None, ppC3, "rwq", pre=rwq_pre)
    rwqstk.close()

    Or = [rtile(m, f"orr{m}") for m in range(HT)]
    with tc.tile_pool(name="pexpR", bufs=2) as pexpR, \
         tc.tile_pool(name="pbcR", bufs=2) as pbcR, \
         tc.tile_pool(name="ptwR", bufs=2) as ptw, \
         tc.tile_pool(name="ppSCr", bufs=2, space="PSUM") as ppSCr, \
         tc.tile_pool(name="ppsum", bufs=2, space="PSUM") as ppsum, \
         tc.tile_pool(name="ppAVr", bufs=1, space="PSUM") as ppAVr:
        for r in range(RH):
            for i in range(3):
                ets = []
                for kvt in range(2):
                    et = pexpR.tile([P, T], BF16, tag=f"re{kvt}",
                                    name=f"re{r}_{i}_{kvt}")
                    for c0 in range(0, T, NCH):
                        psS = ppSCr.tile([P, NCH], F32, tag="psc",
                                         name=f"rsc{r}{i}{kvt}{c0}")
                        for kc in range(2):
                            nc.tensor.matmul(
                                psS[:, :],
                                Kr[i][2 * r + kc][:, kvt * P:(kvt + 1) * P],
                                Qr[2 * r + kc][:, c0:c0 + NCH],
                                start=(kc == 0), stop=(kc == 1))
                        nc.scalar.activation(et[:, c0:c0 + NCH], psS[:],
                                             AF.Exp, scale=1.0 / 16.0)
                    ets.append(et)
                # Z via an all-ones [P,P] matmul: the partition-sum lands
                # broadcast on all 128 partitions, so 1/Z is a full-tile
                # Ln->Exp with no [1,T] rows and no DRAM broadcast.
                rbi = pbcR.tile([P, T], BF16, tag="rbr", name=f"rbr{r}_{i}")
                for c0 in range(0, T, NCH):
                    psZ = ppsum.tile([P, NCH], F32, tag="pz",
                                     name=f"rz{r}{i}{c0}")
                    for kvt in range(2):
                        nc.tensor.matmul(psZ[:, :], allones[:, :],
                                         ets[kvt][:, c0:c0 + NCH],
                                         start=(kvt == 0), stop=(kvt == 1))
                    lnz = pbcR.tile([P, NCH], F32, tag="lnz", bufs=2,
                                    name=f"lnz{r}{i}{c0}")
                    nc.scalar.activation(lnz[:], psZ[:], AF.Ln)
                    nc.scalar.activation(rbi[:, c0:c0 + NCH], lnz[:], AF.Exp,
                                         scale=-1.0)
                for md in range(2):
                    psA = ppAVr.tile([P, T], F32, tag=f"avr{md}",
                                     name=f"rav{r}{i}{md}")
                    col = RDH * r + P * md
                    for c0 in range(0, T, NCH):
                        for kvt in range(2):
                            nc.tensor.matmul(psA[:, c0:c0 + NCH],
                                             Vr[i][kvt][:, col:col + P],
                                             ets[kvt][:, c0:c0 + NCH],
                                             start=(kvt == 0), stop=(kvt == 1))
                    oT = ptw.tile([P, T], BF16, tag=f"ot{md}",
                                  name=f"ot{r}{i}{md}")
                    nc.vector.tensor_copy(out=oT[:], in_=psA[:])
                    dst = Or[2 * r + md]
                    eng_i = nc.vector if i == 2 else nc.gpsimd
                    if i == 0:
                        nc.gpsimd.tensor_mul(dst[:], oT[:], rbi[:])
                    else:
                        tw = ptw.tile([P, T], BF16, tag="tw", name=f"tw{r}{i}{md}")
                        eng_i.tensor_mul(tw[:], oT[:], rbi[:])
                        eng_i.tensor_add(dst[:], dst[:], tw[:])
    cstk2.close()

    # f_w2 preload (covers the merge/FFN span; lives through stage D)
    dstk = ExitStack()
    fw2 = dstk.enter_context(tc.tile_pool(name="fw2", bufs=1))
    fw2_tiles = []
    for k2 in range(4 * HT):
        wt = fw2.tile([P, H], BF16, tag=f"w2s{k2}", name=f"w2s{k2}")
        nc.scalar.dma_start(out=wt[:], in_=D["f_w2T"][k2 * P:(k2 + 1) * P, :])
        fw2_tiles.append(wt)

    x3 = []
    with tc.tile_pool(name="pcomb", bufs=1) as pcomb:
        with tc.tile_pool(name="rwo", bufs=1) as rwo, \
             tc.tile_pool(name="ppC5", bufs=2, space="PSUM") as ppC5:
            comb = []

            def ep_ro(m, ps):
                cm_ = pcomb.tile([P, T], BF16, tag=f"cb{m}", name=f"cb{m}")
                if fl["r_const_zero"] and m % 2 == 0:
                    nc.scalar.activation(cm_[:], ps[:], AF.Copy, bias=0.0,
                                         scale=1.0 / 3.0)
                elif fl["r_const_zero"]:
                    nc.vector.tensor_scalar_mul(cm_[:], ps[:], 1.0 / 3.0)
                else:
                    nc.vector.tensor_scalar(out=cm_[:], in0=ps[:], scalar1=1.0 / 3.0,
                                            scalar2=vcol("r_const", m),
                                            op0=OP.mult, op1=OP.add)
                comb.append(cm_)

            proj("r_woT", Or, ep_ro, rwo, ppC5, "rwo", mh=2, ksplit=6,
                 eng=nc.gpsimd)

        with tc.tile_pool(name="mgw", bufs=1) as mgw, \
             tc.tile_pool(name="pgw", bufs=2) as pgw, \
             tc.tile_pool(name="ptmp", bufs=2) as ptmp, \
             tc.tile_pool(name="ppC6", bufs=3, space="PSUM") as ppC6:
            for half in range(2):
                wb = load_wblocks(mgw, D["mg_wT"], 2 * HT, 4 * P, "mg",
                                  c0=half * 4 * P, bufs=2, eng=nc.scalar)
                for ml in range(4):
                    m = half * 4 + ml
                    ps = ppC6.tile([P, T], F32, tag="pbig", name=f"mgps{m}")
                    steps = [(wb[k][:, ml * P:(ml + 1) * P], x2[k][:])
                             for k in range(HT)]
                    steps += [(wb[HT + k][:, ml * P:(ml + 1) * P], comb[k][:])
                              for k in range(HT)]
                    mm(ps, steps)
                    gw = pgw.tile([P, T], BF16, tag="gw", name=f"gw{m}")
                    nc.scalar.activation(gw[:], ps[:], AF.Sigmoid,
                                         bias=vcol("mg_b", m))
                    eng1 = nc.vector if (m % 8 < 5) else nc.gpsimd
                    eng2 = nc.gpsimd if (m % 8 < 5) else nc.vector
                    d = ptmp.tile([P, T], BF16, tag="d", name=f"d{m}")
                    eng1.tensor_sub(d[:], x2[m][:], comb[m][:])
                    eng1.tensor_mul(d[:], gw[:], d[:])
                    s = ptmp.tile([P, T], BF16, tag="s", name=f"s{m}")
                    eng2.tensor_add(s[:], x2[m][:], comb[m][:])
                    xm = rtile(m, f"x3_{m}")
                    eng1.tensor_add(xm[:], s[:], d[:])
                    x3.append(xm)

    # =========================================================================
    # Stage D: FFN token-half-major + final LN + transpose to [T, H]
    # =========================================================================
    TH = T // 2
    with tc.tile_pool(name="pd", bufs=1) as pd, \
         tc.tile_pool(name="pdh", bufs=1) as pdh, \
         tc.tile_pool(name="pdz", bufs=1) as pdz, \
         tc.tile_pool(name="ppD1", bufs=2, space="PSUM") as ppD1, \
         tc.tile_pool(name="ppD2", bufs=1, space="PSUM") as ppD2, \
         tc.tile_pool(name="ppstD", bufs=2, space="PSUM") as ppstD, \
         tc.tile_pool(name="ppTD", bufs=1, space="PSUM") as ppT:
        for th in range(2):
            c0 = th * TH
            hts = []
            for mg_i in range(8):
                wblk = load_wblocks(pd, D["f_w1T"], HT, 512, "w1s",
                                    c0=mg_i * 512, bufs=2, eng=nc.gpsimd)
                for ml in range(4):
                    m_abs = mg_i * 4 + ml
                    ps = ppD1.tile([P, TH], F32, tag="p1",
                                   name=f"f1ps{th}_{m_abs}")
                    mm(ps, [(wblk[k][:, ml * P:(ml + 1) * P],
                             x3[k][:, c0:c0 + TH]) for k in range(HT)])
                    htile = pdh.tile([P, TH], BF16, tag=f"h{m_abs}",
                                    name=f"h{th}_{m_abs}")
                    nc.scalar.activation(htile[:], ps[:], AF.Gelu,
                                         bias=vcol("f_b1", m_abs))
                    hts.append(htile)
            z4 = [None] * HT
            for grp in range(4):
                pso = [ppD2.tile([P, TH], F32, tag=f"g{j}",
                                 name=f"pso{th}_{grp}_{j}") for j in range(2)]
                for k2 in range(4 * HT):
                    wt = fw2_tiles[k2]
                    for j in range(2):
                        mo = grp * 2 + j
                        nc.tensor.matmul(pso[j][:, :],
                                         wt[:, mo * P:(mo + 1) * P],
                                         hts[k2][:],
                                         start=(k2 == 0), stop=(k2 == 4 * HT - 1))
                for j in range(2):
                    mo = grp * 2 + j
                    zm = pdz.tile([P, TH], BF16, tag="z4", bufs=8,
                                 name=f"z4_{th}_{mo}")
                    if fl["f_b2_zero"]:
                        nc.vector.tensor_add(zm[:], pso[j][:],
                                             x3[mo][:, c0:c0 + TH])
                    else:
                        nc.vector.scalar_tensor_tensor(
                            out=zm[:], in0=pso[j][:], scalar=vcol("f_b2", mo),
                            in1=x3[mo][:, c0:c0 + TH], op0=OP.add, op1=OP.add)
                    z4[mo] = zm
            with tc.tile_pool(name="lnauxD", bufs=1) as lnauxD, \
                 tc.tile_pool(name="bcD", bufs=2) as pbcD, \
                 tc.tile_pool(name="lnoD", bufs=8) as lnoD:
                fin = layer_norm(z4, "n3", (ppstD, lnauxD, pbcD),
                                 lambda k: lnoD.tile([P, TH], BF16, tag="fin",
                                                     name=f"fin{th}_{k}"),
                                 Tn=TH, nch=256)
                for tt in range(TH // P):
                    stg = pdz.tile([P, H], F32, tag="stg", bufs=2,
                                  name=f"stg{th}_{tt}")
                    for hh in range(2):
                        psT4 = ppT.tile([P, 512], BF16, tag="pt",
                                        name=f"pT{th}_{tt}_{hh}")
                        for j in range(4):
                            k2 = hh * 4 + j
                            nc.tensor.transpose(psT4[:, j * P:(j + 1) * P],
                                                fin[k2][:, tt * P:(tt + 1) * P],
                                                identb[:])
                        nc.vector.tensor_copy(
                            out=stg[:, hh * 512:(hh + 1) * 512], in_=psT4[:])
                    row0 = c0 + tt * P
                    nc.sync.dma_start(out=out_d[row0:row0 + P, :], in_=stg[:])
    dstk.close()
    ctx.close()


# =============================================================================
# Host side
# =============================================================================
_CACHE = {}


def _flags(g):
    def zero(a):
        return bool(np.all(a == 0.0))

    fl = {}
    for n in ("sln", "n1", "n2", "n3"):
        fl[f"{n}_trivial"] = bool(np.all(g[f"{n}_g"] == 1.0) and zero(g[f"{n}_b"]))
    fl["outp_b_zero"] = zero(g["outp_b"])
    wq_b, wk_b, wv_b = np.split(g["attn_in_b"], 3, 0)
    fl["a_bq_zero"] = zero(wq_b)
    a_const = wv_b @ g["attn_out_w"].T + g["attn_out_b"]
    fl["a_const_zero"] = zero(a_const)
    rq_b, rk_b, rv_b = np.split(g["retr_in_b"], 3, 0)
    fl["r_bq_zero"] = zero(rq_b)
    r_const = rv_b @ g["retr_out_w"].T + g["retr_out_b"]
    fl["r_const_zero"] = zero(r_const)
    for i in range(3):
        fl[f"c{i}_b2_zero"] = zero(g[f"c{i}_b2"])
    fl["f_b2_zero"] = zero(g["ffn_b2"])
    return fl, a_const, r_const


def _in_maps(g, a_const, r_const):
    import ml_dtypes
    bf16 = ml_dtypes.bfloat16

    def trb(a):
        return np.ascontiguousarray(np.ascontiguousarray(a.T).astype(bf16))

    wq, wk, wv = np.split(g["attn_in_w"], 3, 0)
    rq, rk, rv = np.split(g["retr_in_w"], 3, 0)

    vpack = np.zeros((P, VCOLS), np.float32)

    def setv(nm, vec):
        w = vec.shape[0] // P
        vpack[:, VOFF[nm]:VOFF[nm] + w] = vec.reshape(w, P).T

    setv("A", np.exp(g["A_log"]))
    setv("sg_b", g["sgate_b"])
    setv("Dp1", g["D"] + 1.0)
    setv("outp_b", g["outp_b"])
    setv("a_bq", np.split(g["attn_in_b"], 3, 0)[0])
    setv("a_const", a_const)
    setv("r_bq", np.split(g["retr_in_b"], 3, 0)[0])
    setv("r_const", r_const)
    setv("mg_b", g["mg_b"])
    setv("f_b1", g["ffn_b1"])
    setv("f_b2", g["ffn_b2"])
    for n in ("sln", "n1", "n2", "n3"):
        setv(f"{n}_g", g[f"{n}_g"])
        setv(f"{n}_b", g[f"{n}_b"])
    for i in range(3):
        setv(f"c{i}_b1", g[f"c{i}_b1"])
        setv(f"c{i}_b2", g[f"c{i}_b2"])

    shared = {
        "vpack": vpack,
        "wsgT": trb(g["sgate_w"]), "wBT": trb(g["B_w"]), "wCT": trb(g["C_w"]),
        "outp_wT": trb(g["outp_w"]),
        "a_wqT": trb(wq), "a_wkT": trb(wk), "a_wvT": trb(wv),
        "a_woT": trb(g["attn_out_w"]),
        "r_wqT": trb(rq), "r_wkT": trb(rk), "r_wvT": trb(rv),
        "r_woT": trb(g["retr_out_w"]),
        "mg_wT": trb(g["mg_w"]),
        "f_w1T": trb(g["ffn_w1"]), "f_w2T": trb(g["ffn_w2"]),
    }
    for i in range(3):
        shared[f"c{i}_w1T"] = trb(g[f"c{i}_w1"])
        shared[f"c{i}_w2T"] = trb(g[f"c{i}_w2"])

    in_maps = []
    for b in range(B):
        m = dict(shared)
        m["xT"] = trb(g["x"][b])
        for i in range(3):
            m[f"m{i}T"] = trb(g[f"mem{i}"][b, -256:, :])
        in_maps.append(m)
    return in_maps


def kernel(**inputs):
    g = {k: np.ascontiguousarray(np.asarray(v, dtype=np.float32))
         for k, v in inputs.items()}
    fl, a_const, r_const = _flags(g)

    key = tuple(sorted(fl.items()))
    if key not in _CACHE:
        _CACHE[key] = build_nc(fl)
    nc = _CACHE[key]

    in_maps = _in_maps(g, a_const, r_const)
    trace = os.environ.get("KERNEL_TRACE", "0") == "1"
    res = bass_utils.run_bass_kernel_spmd(nc, in_maps, core_ids=list(range(B)),
                                          trace=trace)
    global LAST_RESULTS
    LAST_RESULTS = res
    out = np.stack([res.results[b]["out"] for b in range(B)], axis=0)
    return out


LAST_RESULTS = None

